# revision 19
# baseline (speedup 1.0000x reference)
"""Trainium2 Bass kernel for the SNN (LIF) network:

    cur1 = x.reshape(B,-1) @ W1.T + b1          (big fp32 matmul, once)
    200 sequential LIF steps on [B,1000] (layer 1), tiny matmul into 5
    outputs per step (layer 2), second LIF on [B,5].

Distribution over 8 cores (v2, scheduling-optimized):
  Phase A: contraction(K)-sharded exact-fp32 matmul (fp16 hi/lo, 3
           passes), split into two hidden halves; each half's partial
           [256, 512] goes through its own ReduceScatter(add) so the
           collective for half 0 overlaps the matmul of half 1. Each
           core ends with its 32-row batch slice of cur1.
  Phase B: per-core LIF layer-1 scan, hidden on partitions
           ([128, 8 chunks x 32 batch] tiles). One custom DVE
           instruction per step: mem' = beta*mem + cur - (mem > 1).
           Pool engine computes spk = (mem > 1) in bf16 {0,1}.
  Phase C: every 4 steps, PE contracts spk (stationary [128, 4*32])
           against W2 chunks split hi/lo bf16 (exact) into PSUM
           [128(sl,b), 5]; Pool adds b2 into cur2s.
  Phase D: layer-2 LIF steps on [32, 5], interleaved on DVE two groups
           behind phase C. spk2 = (mem2 > 1) on Pool at the end.
"""
import os
import sys

if "/opt/trn_rl_repo" not in sys.path:
    sys.path.insert(0, "/opt/trn_rl_repo")

# Profile every core when NTFF tracing is on: exec time = max per-core span
# with aligned starts, instead of core 0's span inflated by the runtime's
# per-device dispatch stagger (~2.3ms) while it waits at the collective.
os.environ.setdefault("BASS_PERFETTO_PROFILE_ALL_CORES", "1")

import numpy as np
import ml_dtypes

# ---------------------------------------------------------------- constants
BETA = 0.95
T = 200
B = 256
NIN = 32000
NH = 1000
NO = 5

N_CORES = 8
KPAD = 32768           # NIN padded to 256*128
KC = KPAD // N_CORES   # 4096 contraction per core
KTILES = KC // 128     # 32
HPAD = 1024            # hidden padded
HHALF = HPAD // 2      # 512 per pipelined half
BLOC = B // N_CORES    # 32 batch rows per core after ReduceScatter
NCHUNK = HPAD // 128   # 8 hidden chunks of 128
G = 4                  # phase-C group size (steps per PE batch)
NGROUP = T // G        # 50
DLAG = 2               # phase-D trails phase-C by this many groups
W1SCALE = 256.0        # W1 pre-scale so the fp16 lo-half stays normal

# ---------------------------------------------------------------- custom op
_LIF_NAME = "LIF_STEP_ANT"


def _register_lif_op():
    from concourse.dve_ops import (
        DveOp, OPS, CUSTOM_DVE_SPECS, _SUB_OPCODE_FOR_NAME, _CUSTOM_DVE_ROW_BASE,
    )
    from concourse.dve_spec import Spec, Src0, Src1, C0, One, lower as dve_lower, _has_src1
    from concourse.dve_uop import DveOpSpec

    for op in OPS:
        if op.name == _LIF_NAME:
            return op
    spec = Spec(
        body=Src0 * C0 + Src1 - (Src0 > One),
        reference=lambda in0, in1, s0: in0 * s0 + in1 - (in0 > 1.0).astype(np.float32),
    )
    if _LIF_NAME not in _SUB_OPCODE_FOR_NAME:
        _SUB_OPCODE_FOR_NAME[_LIF_NAME] = _CUSTOM_DVE_ROW_BASE + len(OPS)
    shas = {}
    for ver in ("v3", "v4"):
        s = DveOpSpec(
            name=_LIF_NAME,
            opcode=_SUB_OPCODE_FOR_NAME[_LIF_NAME],
            uops=dve_lower(spec, ver=ver),
            rd1_en=_has_src1(spec),
        )
        shas[ver] = s.sha(ver)
    op = DveOp(_LIF_NAME, spec, subdim=False, uops_sha=shas)
    OPS.append(op)
    CUSTOM_DVE_SPECS[_LIF_NAME] = op.spec
    return op


# ---------------------------------------------------------------- program
_PROGRAMS = {}  # sim -> (nc, lif_op)


def _build_program(sim=False, dbg=False):
    key = (sim, dbg)
    if key in _PROGRAMS:
        return _PROGRAMS[key]

    import concourse.bass as bass
    import concourse.tile as tile
    from concourse import bacc, mybir
    from concourse.masks import make_identity

    LIF = _register_lif_op()
    f32 = mybir.dt.float32
    bf16 = mybir.dt.bfloat16
    f16 = mybir.dt.float16

    nc = bacc.Bacc("TRN2", target_bir_lowering=False, debug=False,
                   num_devices=1 if sim else N_CORES)

    # inputs (per-core)
    xth_d = nc.dram_tensor("xth", [KTILES, 128, B], f16, kind="ExternalInput").ap()
    xtl_d = nc.dram_tensor("xtl", [KTILES, 128, B], f16, kind="ExternalInput").ap()
    w1h_d = nc.dram_tensor("w1h", [KTILES, 128, HPAD], f16, kind="ExternalInput").ap()
    w1l_d = nc.dram_tensor("w1l", [KTILES, 128, HPAD], f16, kind="ExternalInput").ap()
    b1c_d = nc.dram_tensor("b1c", [128, NCHUNK], f32, kind="ExternalInput").ap()
    w2hi_d = nc.dram_tensor("w2hi", [128, NCHUNK, NO], bf16, kind="ExternalInput").ap()
    w2lo_d = nc.dram_tensor("w2lo", [128, NCHUNK, NO], bf16, kind="ExternalInput").ap()
    b2b_d = nc.dram_tensor("b2b", [128, NO], f32, kind="ExternalInput").ap()
    # outputs (per-core batch slice), free layout = (t, o)
    mem2_d = nc.dram_tensor("mem2rec", [BLOC, T * NO], f32, kind="ExternalOutput").ap()
    spk2_d = nc.dram_tensor("spk2rec", [BLOC, T * NO], f32, kind="ExternalOutput").ap()
    if dbg:
        curdbg_d = nc.dram_tensor("curdbg", [128, NCHUNK * BLOC], f32,
                                  kind="ExternalOutput").ap()
        c2dbg_d = nc.dram_tensor("c2dbg", [BLOC, T * NO], f32,
                                 kind="ExternalOutput").ap()
        gtdbg_d = nc.dram_tensor("gtdbg", [128, NCHUNK * G * BLOC], f32,
                                 kind="ExternalOutput").ap()

    with tile.TileContext(nc) as tc:
        with (
            tc.tile_pool(name="xres", bufs=1) as xres,
            tc.tile_pool(name="win", bufs=8) as wpool,
            tc.tile_pool(name="psA", bufs=2, space="PSUM") as psA,
            tc.tile_pool(name="stage", bufs=1) as stage,
            tc.tile_pool(name="dram", bufs=1, space="DRAM") as dram,
            tc.tile_pool(name="mem", bufs=4) as mpool,
            tc.tile_pool(name="g4", bufs=3) as gpool,
            tc.tile_pool(name="psC", bufs=2, space="PSUM") as psC,
            tc.tile_pool(name="pp", bufs=2) as ppool,
            tc.tile_pool(name="psT", bufs=2, space="PSUM") as psT,
        ):
            # ---------------- phase A (hidden segments, pipelined with RS)
            # x tiles are DMAed just-in-time during segment 0, then stay
            # resident for the later segments.
            xall_h = xres.tile([128, KTILES, B], f16, tag="xah")
            xall_l = xres.tile([128, KTILES, B], f16, tag="xal")
            SEGS = [(0, 512), (512, 256), (768, 256)]
            partials = []
            rs_outs = []
            for hf, (h0, hw) in enumerate(SEGS):
                ps = [psA.tile([128, HHALF], f32, tag=f"ps{mb}", name=f"ps{mb}_{hf}")
                      for mb in range(2)]
                hs = slice(h0, h0 + hw)
                for kt in range(KTILES):
                    if hf == 0:
                        nc.sync.dma_start(xall_h[:, kt, :], xth_d[kt])
                        nc.sync.dma_start(xall_l[:, kt, :], xtl_d[kt])
                    wh_t = wpool.tile([128, HHALF], f16, tag="w1h", name=f"w1h_{hf}_{kt}")
                    nc.sync.dma_start(wh_t[:, 0:hw], w1h_d[kt][:, hs])
                    wl_t = wpool.tile([128, HHALF], f16, tag="w1l", name=f"w1l_{hf}_{kt}")
                    nc.sync.dma_start(wl_t[:, 0:hw], w1l_d[kt][:, hs])
                    last = kt == KTILES - 1
                    for mb in range(2):
                        xh_s = xall_h[:, kt, mb * 128:(mb + 1) * 128]
                        xl_s = xall_l[:, kt, mb * 128:(mb + 1) * 128]
                        out = ps[mb][:, 0:hw]
                        # keep each stationary operand loaded across streams
                        nc.tensor.matmul(out, xh_s, wl_t[:, 0:hw], start=(kt == 0), stop=False)
                        nc.tensor.matmul(out, xh_s, wh_t[:, 0:hw], start=False, stop=False)
                        nc.tensor.matmul(out, xl_s, wh_t[:, 0:hw], start=False, stop=last)
                partial = dram.tile([B, hw], f32, tag=f"partial{hf}", name=f"partial{hf}")
                for mb in range(2):
                    cs = stage.tile([128, hw], f32, tag=f"curp{mb}{hf}", name=f"cs{mb}{hf}")
                    nc.scalar.activation(cs[:], ps[mb][:, 0:hw],
                                         mybir.ActivationFunctionType.Copy,
                                         scale=1.0 / W1SCALE)
                    # chunk the DMA across queues to cut drain latency
                    for q in range(4):
                        qw = hw // 4
                        nc.sync.dma_start(
                            partial[mb * 128:(mb + 1) * 128, q * qw:(q + 1) * qw],
                            cs[:, q * qw:(q + 1) * qw])
                rs_out = dram.tile([BLOC, hw], f32, tag=f"rs{hf}", name=f"rs{hf}")
                if sim:
                    nc.sync.dma_start(rs_out[:], partial[0:BLOC, :])
                else:
                    nc.gpsimd.collective_compute(
                        "ReduceScatter",
                        mybir.AluOpType.add,
                        replica_groups=[list(range(N_CORES))],
                        ins=[partial.opt()],
                        outs=[rs_out.opt()],
                    )
                partials.append(partial)
                rs_outs.append(rs_out)

            # ---------------- transpose to scan layout + fold b1
            # curb[p, c*32 + b] = cur1[b, c*128 + p] + b1[c*128 + p]
            ident = stage.tile([BLOC, BLOC], f32, tag="ident")
            make_identity(nc, ident[:])
            b1t = stage.tile([128, NCHUNK], f32, tag="b1t")
            nc.sync.dma_start(b1t[:], b1c_d[:])
            rsb = [stage.tile([BLOC, hw], f32, tag=f"rsb{hf}", name=f"rsb{hf}")
                   for hf, (h0, hw) in enumerate(SEGS)]
            for hf in range(len(SEGS)):
                nc.sync.dma_start(rsb[hf][:], rs_outs[hf][:])
            curb = stage.tile([128, NCHUNK * BLOC], f32, tag="curb")
            for c in range(NCHUNK):
                hf = next(i for i, (h0, hw) in enumerate(SEGS)
                          if h0 <= c * 128 < h0 + hw)
                ci = (c * 128 - SEGS[hf][0]) // 128
                pt = psT.tile([128, BLOC], f32, tag="pst")
                nc.tensor.transpose(pt[:], rsb[hf][:, ci * 128:(ci + 1) * 128], ident[:])
                nc.scalar.activation(
                    curb[:, c * BLOC:(c + 1) * BLOC], pt[:],
                    mybir.ActivationFunctionType.Identity,
                    bias=b1t[:, c:c + 1], scale=1.0,
                )

            if dbg:
                nc.sync.dma_start(curdbg_d[:], curb[:])

            # ---------------- scan constants
            w2hi_t = stage.tile([128, NCHUNK, NO], bf16, tag="w2hi")
            nc.sync.dma_start(w2hi_t[:], w2hi_d[:])
            w2lo_t = stage.tile([128, NCHUNK, NO], bf16, tag="w2lo")
            nc.sync.dma_start(w2lo_t[:], w2lo_d[:])
            b2b_t = stage.tile([128, NO], f32, tag="b2b")
            nc.sync.dma_start(b2b_t[:], b2b_d[:])
            biasm1 = stage.tile([128, 1], f32, tag="bm1")
            nc.vector.memset(biasm1[:], -1.0)
            zeros_t = stage.tile([128, NCHUNK * BLOC], f32, tag="zeros")
            nc.vector.memset(zeros_t[:], 0.0)
            z32 = stage.tile([BLOC, NO], f32, tag="z32")
            nc.vector.memset(z32[:], 0.0)
            c2r = stage.tile([BLOC, T * NO], f32, tag="c2r")
            mem2r = stage.tile([BLOC, T * NO], f32, tag="mem2r")
            spk2r = stage.tile([BLOC, T * NO], f32, tag="spk2r")

            def d_step(dt):
                """Layer-2 LIF step dt (0-based) on DVE, [32, 5]."""
                in0 = z32[:] if dt == 0 else mem2r[:, (dt - 1) * NO:dt * NO]
                nc.vector._custom_dve(
                    LIF,
                    out=mem2r[:, dt * NO:(dt + 1) * NO],
                    in0=in0,
                    in1=c2r[:, dt * NO:(dt + 1) * NO],
                    s0=BETA,
                )

            # ---------------- phase B/C/D: fused scan
            mem_prev = zeros_t
            gt = None
            for t in range(1, T + 1):
                gi, sl = (t - 1) // G, (t - 1) % G
                if sl == 0:
                    gt = gpool.tile([128, NCHUNK, G * BLOC], bf16, tag="gt")
                m = mpool.tile([128, NCHUNK * BLOC], f32, tag="m")
                nc.vector._custom_dve(LIF, out=m[:], in0=mem_prev[:], in1=curb[:], s0=BETA)
                # g = sign(mem - 1) in {-1,+1} bf16 on ACT; spk=(1+g)/2 folded
                # into the 0.5-scaled W2 and b2eff on the host.
                nc.scalar.activation(
                    gt[:, :, sl * BLOC:(sl + 1) * BLOC],
                    m[:].rearrange("p (c b) -> p c b", b=BLOC),
                    mybir.ActivationFunctionType.Sign, bias=biasm1[:], scale=1.0,
                )
                mem_prev = m
                if sl == G - 1:
                    pc = psC.tile([128, NO], f32, tag="psc")
                    for c in range(NCHUNK):
                        lhs = gt[:, c, :]
                        nc.tensor.matmul(pc[:], lhs, w2hi_t[:, c, :], start=(c == 0), stop=False)
                        nc.tensor.matmul(pc[:], lhs, w2lo_t[:, c, :], start=False,
                                         stop=(c == NCHUNK - 1))
                    # GpSimd can't read PSUM: ACT copies out, GpSimd adds b2eff
                    pcs = ppool.tile([128, NO], f32, tag="pcs")
                    nc.scalar.activation(pcs[:], pc[:],
                                         mybir.ActivationFunctionType.Copy)
                    pcb = ppool.tile([128, NO], f32, tag="pcb")
                    nc.gpsimd.tensor_tensor(
                        pcb[:], pcs[:], b2b_t[:], mybir.AluOpType.add,
                    )
                    # custom-DVE in1 can't take a partition offset: DMA each
                    # sl-row block down to partition base 0 in (t, o) layout
                    for s2 in range(G):
                        dt2 = gi * G + s2
                        nc.sync.dma_start(
                            c2r[:, dt2 * NO:(dt2 + 1) * NO],
                            pcb[s2 * BLOC:(s2 + 1) * BLOC, :],
                        )
                    if dbg and gi == 0:
                        gtf = stage.tile([128, NCHUNK * G * BLOC], f32, tag="gtf")
                        nc.vector.tensor_copy(
                            gtf[:], gt[:].rearrange("p c s -> p (c s)"))
                        nc.sync.dma_start(gtdbg_d[:], gtf[:])
                    if gi >= DLAG:
                        for dt in range((gi - DLAG) * G, (gi - DLAG + 1) * G):
                            d_step(dt)
            for dt in range((NGROUP - DLAG) * G, T):
                d_step(dt)

            # ---------------- spk2 + outputs
            if dbg:
                nc.sync.dma_start(c2dbg_d[:], c2r[:])
            nc.vector.tensor_scalar(spk2r[:], mem2r[:], 1.0, None, mybir.AluOpType.is_gt)
            nc.sync.dma_start(mem2_d[:], mem2r[:])
            nc.sync.dma_start(spk2_d[:], spk2r[:])

    nc.compile()
    _PROGRAMS[key] = (nc, LIF)
    return _PROGRAMS[key]


# ---------------------------------------------------------------- host prep
def _prep_inputs(x, W1, b1, W2, b2):
    f32 = np.float32
    x_flat = np.ascontiguousarray(x.reshape(B, -1).astype(f32, copy=False))  # [256, 32000]
    xT = np.zeros((KPAD, B), f32)
    xT[:NIN] = x_flat.T
    xTh = xT.astype(np.float16)
    xTl = (xT - xTh.astype(f32)).astype(np.float16)
    w1T = np.zeros((KPAD, HPAD), f32)
    w1T[:NIN, :NH] = W1.astype(f32, copy=False).T * W1SCALE
    w1Th = w1T.astype(np.float16)
    w1Tl = (w1T - w1Th.astype(f32)).astype(np.float16)
    b1p = np.full(HPAD, -10.0, f32)
    b1p[:NH] = b1
    b1c = np.ascontiguousarray(b1p.reshape(NCHUNK, 128).T)          # [128, 8]
    W2e = np.zeros((HPAD, NO), f32)
    W2e[:NH] = 0.5 * W2.astype(f32, copy=False).T
    w2stack = np.ascontiguousarray(W2e.reshape(NCHUNK, 128, NO).transpose(1, 0, 2))  # [128,8,5]
    w2hi = w2stack.astype(ml_dtypes.bfloat16)
    w2lo = (w2stack - w2hi.astype(f32)).astype(ml_dtypes.bfloat16)
    b2eff = (b2.astype(f32) + 0.5 * W2.astype(f32).sum(axis=1)).reshape(1, NO)
    b2b = np.ascontiguousarray(np.tile(b2eff, (128, 1)).astype(f32))

    in_maps = []
    for c in range(N_CORES):
        ksl = slice(c * KC, (c + 1) * KC)
        in_maps.append({
            "xth": np.ascontiguousarray(xTh[ksl]).reshape(KTILES, 128, B),
            "xtl": np.ascontiguousarray(xTl[ksl]).reshape(KTILES, 128, B),
            "w1h": np.ascontiguousarray(w1Th[ksl]).reshape(KTILES, 128, HPAD),
            "w1l": np.ascontiguousarray(w1Tl[ksl]).reshape(KTILES, 128, HPAD),
            "b1c": b1c,
            "w2hi": w2hi,
            "w2lo": w2lo,
            "b2b": b2b,
        })
    return in_maps


def _gather(results):
    spk_parts, mem_parts = [], []
    for r in results:
        mem_parts.append(r["mem2rec"].reshape(BLOC, T, NO).transpose(1, 0, 2))
        spk_parts.append(r["spk2rec"].reshape(BLOC, T, NO).transpose(1, 0, 2))
    mem2 = np.concatenate(mem_parts, axis=1).astype(np.float32)  # [200, 256, 5]
    spk2 = np.concatenate(spk_parts, axis=1).astype(np.float32)
    return spk2, mem2


def run_raw(inputs, dbg=False, **kwargs):
    """Build+run; returns BassKernelResults (for profiling from test.py)."""
    from concourse.bass_utils import run_bass_kernel_spmd

    nc, _ = _build_program(dbg=dbg)
    in_maps = _prep_inputs(**inputs)
    return run_bass_kernel_spmd(nc, in_maps, core_ids=list(range(N_CORES)), **kwargs)


def kernel(x, W1, b1, W2, b2):
    res = run_raw(dict(x=x, W1=W1, b1=b1, W2=W2, b2=b2))
    return _gather(res.results)


if __name__ == "__main__":
    rng = np.random.default_rng(0)
    ins = {
        "x": rng.standard_normal((B, 2, 80, 200)).astype(np.float32),
        "W1": rng.uniform(-1, 1, (NH, NIN)).astype(np.float32) / np.sqrt(NIN),
        "b1": rng.uniform(-1, 1, NH).astype(np.float32) / np.sqrt(NIN),
        "W2": rng.uniform(-1, 1, (NO, NH)).astype(np.float32) / np.sqrt(NH),
        "b2": rng.uniform(-1, 1, NO).astype(np.float32) / np.sqrt(NH),
    }
    spk2, mem2 = kernel(**ins)
    print("shapes:", spk2.shape, mem2.shape, spk2.dtype, mem2.dtype)
    print("spk2 mean:", spk2.mean(), "mem2 std:", mem2.std())


# revision 22
# speedup vs baseline: 1.0026x; 1.0026x over previous
"""Trainium2 Bass kernel for the SNN (LIF) network:

    cur1 = x.reshape(B,-1) @ W1.T + b1          (big fp32 matmul, once)
    200 sequential LIF steps on [B,1000] (layer 1), tiny matmul into 5
    outputs per step (layer 2), second LIF on [B,5].

Distribution over 8 cores (v2, scheduling-optimized):
  Phase A: contraction(K)-sharded exact-fp32 matmul (fp16 hi/lo, 3
           passes), split into two hidden halves; each half's partial
           [256, 512] goes through its own ReduceScatter(add) so the
           collective for half 0 overlaps the matmul of half 1. Each
           core ends with its 32-row batch slice of cur1.
  Phase B: per-core LIF layer-1 scan, hidden on partitions
           ([128, 8 chunks x 32 batch] tiles). One custom DVE
           instruction per step: mem' = beta*mem + cur - (mem > 1).
           Pool engine computes spk = (mem > 1) in bf16 {0,1}.
  Phase C: every 4 steps, PE contracts spk (stationary [128, 4*32])
           against W2 chunks split hi/lo bf16 (exact) into PSUM
           [128(sl,b), 5]; Pool adds b2 into cur2s.
  Phase D: layer-2 LIF steps on [32, 5], interleaved on DVE two groups
           behind phase C. spk2 = (mem2 > 1) on Pool at the end.
"""
import os
import sys

if "/opt/trn_rl_repo" not in sys.path:
    sys.path.insert(0, "/opt/trn_rl_repo")

# Profile every core when NTFF tracing is on: exec time = max per-core span
# with aligned starts, instead of core 0's span inflated by the runtime's
# per-device dispatch stagger (~2.3ms) while it waits at the collective.
os.environ.setdefault("BASS_PERFETTO_PROFILE_ALL_CORES", "1")

import numpy as np
import ml_dtypes

# ---------------------------------------------------------------- constants
BETA = 0.95
T = 200
B = 256
NIN = 32000
NH = 1000
NO = 5

N_CORES = 8
KPAD = 32768           # NIN padded to 256*128
KC = KPAD // N_CORES   # 4096 contraction per core
KTILES = KC // 128     # 32
HPAD = 1024            # hidden padded
HHALF = HPAD // 2      # 512 per pipelined half
BLOC = B // N_CORES    # 32 batch rows per core after ReduceScatter
NCHUNK = HPAD // 128   # 8 hidden chunks of 128
G = 4                  # phase-C group size (steps per PE batch)
NGROUP = T // G        # 50
DLAG = 2               # phase-D trails phase-C by this many groups
W1SCALE = 256.0        # W1 pre-scale so the fp16 lo-half stays normal

# ---------------------------------------------------------------- custom op
_LIF_NAME = "LIF_STEP_ANT"


def _register_lif_op():
    from concourse.dve_ops import (
        DveOp, OPS, CUSTOM_DVE_SPECS, _SUB_OPCODE_FOR_NAME, _CUSTOM_DVE_ROW_BASE,
    )
    from concourse.dve_spec import Spec, Src0, Src1, C0, One, lower as dve_lower, _has_src1
    from concourse.dve_uop import DveOpSpec

    for op in OPS:
        if op.name == _LIF_NAME:
            return op
    spec = Spec(
        body=Src0 * C0 + Src1 - (Src0 > One),
        reference=lambda in0, in1, s0: in0 * s0 + in1 - (in0 > 1.0).astype(np.float32),
    )
    if _LIF_NAME not in _SUB_OPCODE_FOR_NAME:
        _SUB_OPCODE_FOR_NAME[_LIF_NAME] = _CUSTOM_DVE_ROW_BASE + len(OPS)
    shas = {}
    for ver in ("v3", "v4"):
        s = DveOpSpec(
            name=_LIF_NAME,
            opcode=_SUB_OPCODE_FOR_NAME[_LIF_NAME],
            uops=dve_lower(spec, ver=ver),
            rd1_en=_has_src1(spec),
        )
        shas[ver] = s.sha(ver)
    op = DveOp(_LIF_NAME, spec, subdim=False, uops_sha=shas)
    OPS.append(op)
    CUSTOM_DVE_SPECS[_LIF_NAME] = op.spec
    return op


# ---------------------------------------------------------------- program
_PROGRAMS = {}  # sim -> (nc, lif_op)


def _build_program(sim=False, dbg=False):
    key = (sim, dbg)
    if key in _PROGRAMS:
        return _PROGRAMS[key]

    import concourse.bass as bass
    import concourse.tile as tile
    from concourse import bacc, mybir
    from concourse.masks import make_identity

    LIF = _register_lif_op()
    f32 = mybir.dt.float32
    bf16 = mybir.dt.bfloat16
    f16 = mybir.dt.float16

    nc = bacc.Bacc("TRN2", target_bir_lowering=False, debug=False,
                   num_devices=1 if sim else N_CORES)

    # inputs (per-core)
    xth_d = nc.dram_tensor("xth", [KTILES, 128, B], f16, kind="ExternalInput").ap()
    xtl_d = nc.dram_tensor("xtl", [KTILES, 128, B], f16, kind="ExternalInput").ap()
    # W1 halves pre-sliced per hidden segment on the host so every DMA read
    # is fully contiguous (strided reads starve the PE).
    SEGW = [512, 256, 256]
    w1h_d = [nc.dram_tensor(f"w1h{i}", [KTILES, 128, w], f16, kind="ExternalInput").ap()
             for i, w in enumerate(SEGW)]
    w1l_d = [nc.dram_tensor(f"w1l{i}", [KTILES, 128, w], f16, kind="ExternalInput").ap()
             for i, w in enumerate(SEGW)]
    b1c_d = nc.dram_tensor("b1c", [128, NCHUNK], f32, kind="ExternalInput").ap()
    w2hi_d = nc.dram_tensor("w2hi", [128, NCHUNK, NO], bf16, kind="ExternalInput").ap()
    w2lo_d = nc.dram_tensor("w2lo", [128, NCHUNK, NO], bf16, kind="ExternalInput").ap()
    b2b_d = nc.dram_tensor("b2b", [128, NO], f32, kind="ExternalInput").ap()
    # outputs (per-core batch slice), free layout = (t, o)
    mem2_d = nc.dram_tensor("mem2rec", [BLOC, T * NO], f32, kind="ExternalOutput").ap()
    spk2_d = nc.dram_tensor("spk2rec", [BLOC, T * NO], f32, kind="ExternalOutput").ap()
    if dbg:
        curdbg_d = nc.dram_tensor("curdbg", [128, NCHUNK * BLOC], f32,
                                  kind="ExternalOutput").ap()
        c2dbg_d = nc.dram_tensor("c2dbg", [BLOC, T * NO], f32,
                                 kind="ExternalOutput").ap()
        gtdbg_d = nc.dram_tensor("gtdbg", [128, NCHUNK * G * BLOC], f32,
                                 kind="ExternalOutput").ap()

    with tile.TileContext(nc) as tc:
        with (
            tc.tile_pool(name="xres", bufs=1) as xres,
            tc.tile_pool(name="win", bufs=8) as wpool,
            tc.tile_pool(name="psA", bufs=2, space="PSUM") as psA,
            tc.tile_pool(name="stage", bufs=1) as stage,
            tc.tile_pool(name="dram", bufs=1, space="DRAM") as dram,
            tc.tile_pool(name="mem", bufs=4) as mpool,
            tc.tile_pool(name="g4", bufs=3) as gpool,
            tc.tile_pool(name="psC", bufs=2, space="PSUM") as psC,
            tc.tile_pool(name="pp", bufs=2) as ppool,
            tc.tile_pool(name="psT", bufs=2, space="PSUM") as psT,
        ):
            # ---------------- phase A (hidden segments, pipelined with RS)
            # x tiles are DMAed just-in-time during segment 0, then stay
            # resident for the later segments.
            xall_h = xres.tile([128, KTILES, B], f16, tag="xah")
            xall_l = xres.tile([128, KTILES, B], f16, tag="xal")
            SEGS = [(0, 512), (512, 256), (768, 256)]
            partials = []
            rs_outs = []
            for hf, (h0, hw) in enumerate(SEGS):
                ps = [psA.tile([128, HHALF], f32, tag=f"ps{mb}", name=f"ps{mb}_{hf}")
                      for mb in range(2)]
                for kt in range(KTILES):
                    if hf == 0:
                        nc.sync.dma_start(xall_h[:, kt, :], xth_d[kt])
                        nc.sync.dma_start(xall_l[:, kt, :], xtl_d[kt])
                    wh_t = wpool.tile([128, HHALF], f16, tag="w1h", name=f"w1h_{hf}_{kt}")
                    nc.sync.dma_start(wh_t[:, 0:hw], w1h_d[hf][kt])
                    wl_t = wpool.tile([128, HHALF], f16, tag="w1l", name=f"w1l_{hf}_{kt}")
                    nc.sync.dma_start(wl_t[:, 0:hw], w1l_d[hf][kt])
                    last = kt == KTILES - 1
                    for mb in range(2):
                        xh_s = xall_h[:, kt, mb * 128:(mb + 1) * 128]
                        xl_s = xall_l[:, kt, mb * 128:(mb + 1) * 128]
                        out = ps[mb][:, 0:hw]
                        # keep each stationary operand loaded across streams
                        nc.tensor.matmul(out, xh_s, wl_t[:, 0:hw], start=(kt == 0), stop=False)
                        nc.tensor.matmul(out, xh_s, wh_t[:, 0:hw], start=False, stop=False)
                        nc.tensor.matmul(out, xl_s, wh_t[:, 0:hw], start=False, stop=last)
                partial = dram.tile([B, hw], f32, tag=f"partial{hf}", name=f"partial{hf}")
                for mb in range(2):
                    cs = stage.tile([128, hw], f32, tag=f"curp{mb}{hf}", name=f"cs{mb}{hf}")
                    nc.scalar.activation(cs[:], ps[mb][:, 0:hw],
                                         mybir.ActivationFunctionType.Copy,
                                         scale=1.0 / W1SCALE)
                    # chunk the DMA across queues to cut drain latency
                    for q in range(4):
                        qw = hw // 4
                        nc.sync.dma_start(
                            partial[mb * 128:(mb + 1) * 128, q * qw:(q + 1) * qw],
                            cs[:, q * qw:(q + 1) * qw])
                rs_out = dram.tile([BLOC, hw], f32, tag=f"rs{hf}", name=f"rs{hf}")
                if sim:
                    nc.sync.dma_start(rs_out[:], partial[0:BLOC, :])
                else:
                    nc.gpsimd.collective_compute(
                        "ReduceScatter",
                        mybir.AluOpType.add,
                        replica_groups=[list(range(N_CORES))],
                        ins=[partial.opt()],
                        outs=[rs_out.opt()],
                    )
                partials.append(partial)
                rs_outs.append(rs_out)

            # ---------------- transpose to scan layout + fold b1
            # curb[p, c*32 + b] = cur1[b, c*128 + p] + b1[c*128 + p]
            ident = stage.tile([BLOC, BLOC], f32, tag="ident")
            make_identity(nc, ident[:])
            b1t = stage.tile([128, NCHUNK], f32, tag="b1t")
            nc.sync.dma_start(b1t[:], b1c_d[:])
            rsb = [stage.tile([BLOC, hw], f32, tag=f"rsb{hf}", name=f"rsb{hf}")
                   for hf, (h0, hw) in enumerate(SEGS)]
            for hf in range(len(SEGS)):
                nc.sync.dma_start(rsb[hf][:], rs_outs[hf][:])
            curb = stage.tile([128, NCHUNK * BLOC], f32, tag="curb")
            for c in range(NCHUNK):
                hf = next(i for i, (h0, hw) in enumerate(SEGS)
                          if h0 <= c * 128 < h0 + hw)
                ci = (c * 128 - SEGS[hf][0]) // 128
                pt = psT.tile([128, BLOC], f32, tag="pst")
                nc.tensor.transpose(pt[:], rsb[hf][:, ci * 128:(ci + 1) * 128], ident[:])
                nc.scalar.activation(
                    curb[:, c * BLOC:(c + 1) * BLOC], pt[:],
                    mybir.ActivationFunctionType.Identity,
                    bias=b1t[:, c:c + 1], scale=1.0,
                )

            if dbg:
                nc.sync.dma_start(curdbg_d[:], curb[:])

            # ---------------- scan constants
            w2hi_t = stage.tile([128, NCHUNK, NO], bf16, tag="w2hi")
            nc.sync.dma_start(w2hi_t[:], w2hi_d[:])
            w2lo_t = stage.tile([128, NCHUNK, NO], bf16, tag="w2lo")
            nc.sync.dma_start(w2lo_t[:], w2lo_d[:])
            b2b_t = stage.tile([128, NO], f32, tag="b2b")
            nc.sync.dma_start(b2b_t[:], b2b_d[:])
            biasm1 = stage.tile([128, 1], f32, tag="bm1")
            nc.vector.memset(biasm1[:], -1.0)
            zeros_t = stage.tile([128, NCHUNK * BLOC], f32, tag="zeros")
            nc.vector.memset(zeros_t[:], 0.0)
            z32 = stage.tile([BLOC, NO], f32, tag="z32")
            nc.vector.memset(z32[:], 0.0)
            c2r = stage.tile([BLOC, T * NO], f32, tag="c2r")
            mem2r = stage.tile([BLOC, T * NO], f32, tag="mem2r")
            spk2r = stage.tile([BLOC, T * NO], f32, tag="spk2r")

            def d_step(dt):
                """Layer-2 LIF step dt (0-based) on DVE, [32, 5]."""
                in0 = z32[:] if dt == 0 else mem2r[:, (dt - 1) * NO:dt * NO]
                nc.vector._custom_dve(
                    LIF,
                    out=mem2r[:, dt * NO:(dt + 1) * NO],
                    in0=in0,
                    in1=c2r[:, dt * NO:(dt + 1) * NO],
                    s0=BETA,
                )

            # ---------------- phase B/C/D: fused scan
            mem_prev = zeros_t
            gt = None
            for t in range(1, T + 1):
                gi, sl = (t - 1) // G, (t - 1) % G
                if sl == 0:
                    gt = gpool.tile([128, NCHUNK, G * BLOC], bf16, tag="gt")
                m = mpool.tile([128, NCHUNK * BLOC], f32, tag="m")
                nc.vector._custom_dve(LIF, out=m[:], in0=mem_prev[:], in1=curb[:], s0=BETA)
                # g = sign(mem - 1) in {-1,+1} bf16 on ACT; spk=(1+g)/2 folded
                # into the 0.5-scaled W2 and b2eff on the host.
                nc.scalar.activation(
                    gt[:, :, sl * BLOC:(sl + 1) * BLOC],
                    m[:].rearrange("p (c b) -> p c b", b=BLOC),
                    mybir.ActivationFunctionType.Sign, bias=biasm1[:], scale=1.0,
                )
                mem_prev = m
                if sl == G - 1:
                    pc = psC.tile([128, NO], f32, tag="psc")
                    for c in range(NCHUNK):
                        lhs = gt[:, c, :]
                        nc.tensor.matmul(pc[:], lhs, w2hi_t[:, c, :], start=(c == 0), stop=False)
                        nc.tensor.matmul(pc[:], lhs, w2lo_t[:, c, :], start=False,
                                         stop=(c == NCHUNK - 1))
                    # GpSimd can't read PSUM: ACT copies out, GpSimd adds b2eff
                    pcs = ppool.tile([128, NO], f32, tag="pcs")
                    nc.scalar.activation(pcs[:], pc[:],
                                         mybir.ActivationFunctionType.Copy)
                    pcb = ppool.tile([128, NO], f32, tag="pcb")
                    nc.gpsimd.tensor_tensor(
                        pcb[:], pcs[:], b2b_t[:], mybir.AluOpType.add,
                    )
                    # custom-DVE in1 can't take a partition offset: DMA each
                    # sl-row block down to partition base 0 in (t, o) layout
                    for s2 in range(G):
                        dt2 = gi * G + s2
                        nc.sync.dma_start(
                            c2r[:, dt2 * NO:(dt2 + 1) * NO],
                            pcb[s2 * BLOC:(s2 + 1) * BLOC, :],
                        )
                    if dbg and gi == 0:
                        gtf = stage.tile([128, NCHUNK * G * BLOC], f32, tag="gtf")
                        nc.vector.tensor_copy(
                            gtf[:], gt[:].rearrange("p c s -> p (c s)"))
                        nc.sync.dma_start(gtdbg_d[:], gtf[:])
                    if gi >= DLAG:
                        for dt in range((gi - DLAG) * G, (gi - DLAG + 1) * G):
                            d_step(dt)
            for dt in range((NGROUP - DLAG) * G, T):
                d_step(dt)

            # ---------------- spk2 + outputs
            if dbg:
                nc.sync.dma_start(c2dbg_d[:], c2r[:])
            nc.vector.tensor_scalar(spk2r[:], mem2r[:], 1.0, None, mybir.AluOpType.is_gt)
            nc.sync.dma_start(mem2_d[:], mem2r[:])
            nc.sync.dma_start(spk2_d[:], spk2r[:])

    nc.compile()
    _PROGRAMS[key] = (nc, LIF)
    return _PROGRAMS[key]


# ---------------------------------------------------------------- host prep
def _prep_inputs(x, W1, b1, W2, b2):
    f32 = np.float32
    x_flat = np.ascontiguousarray(x.reshape(B, -1).astype(f32, copy=False))  # [256, 32000]
    xT = np.zeros((KPAD, B), f32)
    xT[:NIN] = x_flat.T
    xTh = xT.astype(np.float16)
    xTl = (xT - xTh.astype(f32)).astype(np.float16)
    w1T = np.zeros((KPAD, HPAD), f32)
    w1T[:NIN, :NH] = W1.astype(f32, copy=False).T * W1SCALE
    w1Th = w1T.astype(np.float16)
    w1Tl = (w1T - w1Th.astype(f32)).astype(np.float16)
    b1p = np.full(HPAD, -10.0, f32)
    b1p[:NH] = b1
    b1c = np.ascontiguousarray(b1p.reshape(NCHUNK, 128).T)          # [128, 8]
    W2e = np.zeros((HPAD, NO), f32)
    W2e[:NH] = 0.5 * W2.astype(f32, copy=False).T
    w2stack = np.ascontiguousarray(W2e.reshape(NCHUNK, 128, NO).transpose(1, 0, 2))  # [128,8,5]
    w2hi = w2stack.astype(ml_dtypes.bfloat16)
    w2lo = (w2stack - w2hi.astype(f32)).astype(ml_dtypes.bfloat16)
    b2eff = (b2.astype(f32) + 0.5 * W2.astype(f32).sum(axis=1)).reshape(1, NO)
    b2b = np.ascontiguousarray(np.tile(b2eff, (128, 1)).astype(f32))

    segs = [(0, 512), (512, 256), (768, 256)]
    in_maps = []
    for c in range(N_CORES):
        ksl = slice(c * KC, (c + 1) * KC)
        wh = w1Th[ksl].reshape(KTILES, 128, HPAD)
        wl = w1Tl[ksl].reshape(KTILES, 128, HPAD)
        m = {
            "xth": np.ascontiguousarray(xTh[ksl]).reshape(KTILES, 128, B),
            "xtl": np.ascontiguousarray(xTl[ksl]).reshape(KTILES, 128, B),
            "b1c": b1c,
            "w2hi": w2hi,
            "w2lo": w2lo,
            "b2b": b2b,
        }
        for i, (h0, hw) in enumerate(segs):
            m[f"w1h{i}"] = np.ascontiguousarray(wh[:, :, h0:h0 + hw])
            m[f"w1l{i}"] = np.ascontiguousarray(wl[:, :, h0:h0 + hw])
        in_maps.append(m)
    return in_maps


def _gather(results):
    spk_parts, mem_parts = [], []
    for r in results:
        mem_parts.append(r["mem2rec"].reshape(BLOC, T, NO).transpose(1, 0, 2))
        spk_parts.append(r["spk2rec"].reshape(BLOC, T, NO).transpose(1, 0, 2))
    mem2 = np.concatenate(mem_parts, axis=1).astype(np.float32)  # [200, 256, 5]
    spk2 = np.concatenate(spk_parts, axis=1).astype(np.float32)
    return spk2, mem2


def run_raw(inputs, dbg=False, **kwargs):
    """Build+run; returns BassKernelResults (for profiling from test.py)."""
    from concourse.bass_utils import run_bass_kernel_spmd

    nc, _ = _build_program(dbg=dbg)
    in_maps = _prep_inputs(**inputs)
    return run_bass_kernel_spmd(nc, in_maps, core_ids=list(range(N_CORES)), **kwargs)


def kernel(x, W1, b1, W2, b2):
    res = run_raw(dict(x=x, W1=W1, b1=b1, W2=W2, b2=b2))
    return _gather(res.results)


if __name__ == "__main__":
    rng = np.random.default_rng(0)
    ins = {
        "x": rng.standard_normal((B, 2, 80, 200)).astype(np.float32),
        "W1": rng.uniform(-1, 1, (NH, NIN)).astype(np.float32) / np.sqrt(NIN),
        "b1": rng.uniform(-1, 1, NH).astype(np.float32) / np.sqrt(NIN),
        "W2": rng.uniform(-1, 1, (NO, NH)).astype(np.float32) / np.sqrt(NH),
        "b2": rng.uniform(-1, 1, NO).astype(np.float32) / np.sqrt(NH),
    }
    spk2, mem2 = kernel(**ins)
    print("shapes:", spk2.shape, mem2.shape, spk2.dtype, mem2.dtype)
    print("spk2 mean:", spk2.mean(), "mem2 std:", mem2.std())


# revision 27
# speedup vs baseline: 1.2071x; 1.2039x over previous
"""Trainium2 Bass kernel for the SNN (LIF) network:

    cur1 = x.reshape(B,-1) @ W1.T + b1          (big fp32 matmul, once)
    200 sequential LIF steps on [B,1000] (layer 1), tiny matmul into 5
    outputs per step (layer 2), second LIF on [B,5].

Distribution over 8 cores (v2, scheduling-optimized):
  Phase A: contraction(K)-sharded exact-fp32 matmul (fp16 hi/lo, 3
           passes), split into two hidden halves; each half's partial
           [256, 512] goes through its own ReduceScatter(add) so the
           collective for half 0 overlaps the matmul of half 1. Each
           core ends with its 32-row batch slice of cur1.
  Phase B: per-core LIF layer-1 scan, hidden on partitions
           ([128, 8 chunks x 32 batch] tiles). One custom DVE
           instruction per step: mem' = beta*mem + cur - (mem > 1).
           Pool engine computes spk = (mem > 1) in bf16 {0,1}.
  Phase C: every 4 steps, PE contracts spk (stationary [128, 4*32])
           against W2 chunks split hi/lo bf16 (exact) into PSUM
           [128(sl,b), 5]; Pool adds b2 into cur2s.
  Phase D: layer-2 LIF steps on [32, 5], interleaved on DVE two groups
           behind phase C. spk2 = (mem2 > 1) on Pool at the end.
"""
import os
import sys

if "/opt/trn_rl_repo" not in sys.path:
    sys.path.insert(0, "/opt/trn_rl_repo")

# Profile every core when NTFF tracing is on: exec time = max per-core span
# with aligned starts, instead of core 0's span inflated by the runtime's
# per-device dispatch stagger (~2.3ms) while it waits at the collective.
os.environ.setdefault("BASS_PERFETTO_PROFILE_ALL_CORES", "1")

import numpy as np
import ml_dtypes

# ---------------------------------------------------------------- constants
BETA = 0.95
T = 200
B = 256
NIN = 32000
NH = 1000
NO = 5

N_CORES = 8
KPAD = 32768           # NIN padded to 256*128
KC = KPAD // N_CORES   # 4096 contraction per core
KTILES = KC // 128     # 32
HPAD = 1024            # hidden padded
HHALF = HPAD // 2      # 512 per pipelined half
BLOC = B // N_CORES    # 32 batch rows per core after ReduceScatter
NCHUNK = HPAD // 128   # 8 hidden chunks of 128
G = 4                  # phase-C group size (steps per PE batch)
NGROUP = T // G        # 50
DLAG = 2               # phase-D trails phase-C by this many groups
W1SCALE = 256.0        # W1 pre-scale so the fp16 lo-half stays normal

# ---------------------------------------------------------------- custom op
_LIF_NAME = "LIF_STEP_ANT"


def _register_lif_op():
    from concourse.dve_ops import (
        DveOp, OPS, CUSTOM_DVE_SPECS, _SUB_OPCODE_FOR_NAME, _CUSTOM_DVE_ROW_BASE,
    )
    from concourse.dve_spec import Spec, Src0, Src1, C0, One, lower as dve_lower, _has_src1
    from concourse.dve_uop import DveOpSpec

    for op in OPS:
        if op.name == _LIF_NAME:
            return op
    spec = Spec(
        body=Src0 * C0 + Src1 - (Src0 > One),
        reference=lambda in0, in1, s0: in0 * s0 + in1 - (in0 > 1.0).astype(np.float32),
    )
    if _LIF_NAME not in _SUB_OPCODE_FOR_NAME:
        _SUB_OPCODE_FOR_NAME[_LIF_NAME] = _CUSTOM_DVE_ROW_BASE + len(OPS)
    shas = {}
    for ver in ("v3", "v4"):
        s = DveOpSpec(
            name=_LIF_NAME,
            opcode=_SUB_OPCODE_FOR_NAME[_LIF_NAME],
            uops=dve_lower(spec, ver=ver),
            rd1_en=_has_src1(spec),
        )
        shas[ver] = s.sha(ver)
    op = DveOp(_LIF_NAME, spec, subdim=False, uops_sha=shas)
    OPS.append(op)
    CUSTOM_DVE_SPECS[_LIF_NAME] = op.spec
    return op


# ---------------------------------------------------------------- program
_PROGRAMS = {}  # sim -> (nc, lif_op)


def _build_program(sim=False, dbg=False):
    key = (sim, dbg)
    if key in _PROGRAMS:
        return _PROGRAMS[key]

    import concourse.bass as bass
    import concourse.tile as tile
    from concourse import bacc, mybir
    from concourse.masks import make_identity

    LIF = _register_lif_op()
    f32 = mybir.dt.float32
    bf16 = mybir.dt.bfloat16
    f16 = mybir.dt.float16

    nc = bacc.Bacc("TRN2", target_bir_lowering=False, debug=False,
                   num_devices=1 if sim else N_CORES)

    # inputs (per-core)
    xth_d = nc.dram_tensor("xth", [KTILES, 128, B], f16, kind="ExternalInput").ap()
    xtl_d = nc.dram_tensor("xtl", [KTILES, 128, B], f16, kind="ExternalInput").ap()
    w1h_d = nc.dram_tensor("w1h", [KTILES, 128, HPAD], f16, kind="ExternalInput").ap()
    w1l_d = nc.dram_tensor("w1l", [KTILES, 128, HPAD], f16, kind="ExternalInput").ap()
    b1c_d = nc.dram_tensor("b1c", [128, NCHUNK], f32, kind="ExternalInput").ap()
    w2hi_d = nc.dram_tensor("w2hi", [128, NCHUNK, NO], bf16, kind="ExternalInput").ap()
    w2lo_d = nc.dram_tensor("w2lo", [128, NCHUNK, NO], bf16, kind="ExternalInput").ap()
    b2b_d = nc.dram_tensor("b2b", [128, NO], f32, kind="ExternalInput").ap()
    # outputs (per-core batch slice), free layout = (t, o)
    mem2_d = nc.dram_tensor("mem2rec", [BLOC, T * NO], f32, kind="ExternalOutput").ap()
    spk2_d = nc.dram_tensor("spk2rec", [BLOC, T * NO], f32, kind="ExternalOutput").ap()
    if dbg:
        curdbg_d = nc.dram_tensor("curdbg", [128, NCHUNK * BLOC], f32,
                                  kind="ExternalOutput").ap()
        c2dbg_d = nc.dram_tensor("c2dbg", [BLOC, T * NO], f32,
                                 kind="ExternalOutput").ap()
        gtdbg_d = nc.dram_tensor("gtdbg", [128, NCHUNK * G * BLOC], f32,
                                 kind="ExternalOutput").ap()

    with tile.TileContext(nc) as tc:
        with (
            tc.tile_pool(name="xres", bufs=1) as xres,
            tc.tile_pool(name="win", bufs=8) as wpool,
            tc.tile_pool(name="psA", bufs=1, space="PSUM") as psA,
            tc.tile_pool(name="stage", bufs=1) as stage,
            tc.tile_pool(name="dram", bufs=1, space="DRAM") as dram,
            tc.tile_pool(name="mem", bufs=4) as mpool,
            tc.tile_pool(name="g4", bufs=3) as gpool,
            tc.tile_pool(name="psC", bufs=2, space="PSUM") as psC,
            tc.tile_pool(name="pp", bufs=2) as ppool,
            tc.tile_pool(name="psT", bufs=2, space="PSUM") as psT,
        ):
            # ---------------- phase A, split along K for a pipelined RS
            # Full-row W DMAs ([128,1024], 256KB contiguous) keep the DMA
            # queues at peak rate; the PSUM accumulation stops at the K
            # midpoint so the first half's ReduceScatter overlaps the second
            # half's matmuls. rs = rs_a + rs_b afterwards on DVE.
            xall_h = xres.tile([128, KTILES, B], f16, tag="xah")
            xall_l = xres.tile([128, KTILES, B], f16, tag="xal")
            KSPLIT = [(0, KTILES // 2), (KTILES // 2, KTILES)]
            rs_outs = []
            for kk, (ka, kb) in enumerate(KSPLIT):
                ps = [[psA.tile([128, 512], f32, tag=f"ps{mb}{nb}",
                                name=f"ps{mb}{nb}_{kk}")
                       for nb in range(2)] for mb in range(2)]
                for kt in range(ka, kb):
                    nc.sync.dma_start(xall_h[:, kt, :], xth_d[kt])
                    nc.sync.dma_start(xall_l[:, kt, :], xtl_d[kt])
                    wh_t = wpool.tile([128, HPAD], f16, tag="w1h")
                    nc.sync.dma_start(wh_t[:], w1h_d[kt])
                    wl_t = wpool.tile([128, HPAD], f16, tag="w1l")
                    nc.sync.dma_start(wl_t[:], w1l_d[kt])
                    last = kt == kb - 1
                    for mb in range(2):
                        xh_s = xall_h[:, kt, mb * 128:(mb + 1) * 128]
                        xl_s = xall_l[:, kt, mb * 128:(mb + 1) * 128]
                        # keep each stationary operand loaded across streams
                        for nb in range(2):
                            out = ps[mb][nb][:]
                            nc.tensor.matmul(out, xh_s, wl_t[:, nb * 512:(nb + 1) * 512],
                                             start=(kt == ka), stop=False)
                            nc.tensor.matmul(out, xh_s, wh_t[:, nb * 512:(nb + 1) * 512],
                                             start=False, stop=False)
                        for nb in range(2):
                            nc.tensor.matmul(ps[mb][nb][:], xl_s,
                                             wh_t[:, nb * 512:(nb + 1) * 512],
                                             start=False, stop=last)
                partial = dram.tile([B, HPAD], f32, tag=f"partial{kk}",
                                    name=f"partial{kk}")
                for mb in range(2):
                    cs = stage.tile([128, HPAD], f32, tag=f"curp{mb}",
                                    name=f"cs{mb}{kk}")
                    for nb in range(2):
                        nc.scalar.activation(
                            cs[:, nb * 512:(nb + 1) * 512], ps[mb][nb][:],
                            mybir.ActivationFunctionType.Copy, scale=1.0 / W1SCALE)
                    # chunk the DMA across queues to cut drain latency
                    for q in range(4):
                        nc.sync.dma_start(
                            partial[mb * 128:(mb + 1) * 128, q * 256:(q + 1) * 256],
                            cs[:, q * 256:(q + 1) * 256])
                rs_out = dram.tile([BLOC, HPAD], f32, tag=f"rs{kk}", name=f"rs{kk}")
                if sim:
                    nc.sync.dma_start(rs_out[:], partial[0:BLOC, :])
                else:
                    nc.gpsimd.collective_compute(
                        "ReduceScatter",
                        mybir.AluOpType.add,
                        replica_groups=[list(range(N_CORES))],
                        ins=[partial.opt()],
                        outs=[rs_out.opt()],
                    )
                rs_outs.append(rs_out)

            # ---------------- transpose to scan layout + fold b1
            # curb[p, c*32 + b] = cur1[b, c*128 + p] + b1[c*128 + p]
            ident = stage.tile([BLOC, BLOC], f32, tag="ident")
            make_identity(nc, ident[:])
            b1t = stage.tile([128, NCHUNK], f32, tag="b1t")
            nc.sync.dma_start(b1t[:], b1c_d[:])
            rsb = [stage.tile([BLOC, HPAD], f32, tag=f"rsb{kk}", name=f"rsb{kk}")
                   for kk in range(2)]
            for kk in range(2):
                nc.sync.dma_start(rsb[kk][:], rs_outs[kk][:])
            rsv = stage.tile([BLOC, HPAD], f32, tag="rsv")
            nc.vector.tensor_tensor(rsv[:], rsb[0][:], rsb[1][:],
                                    mybir.AluOpType.add)
            curb = stage.tile([128, NCHUNK * BLOC], f32, tag="curb")
            for c in range(NCHUNK):
                pt = psT.tile([128, BLOC], f32, tag="pst")
                nc.tensor.transpose(pt[:], rsv[:, c * 128:(c + 1) * 128], ident[:])
                nc.scalar.activation(
                    curb[:, c * BLOC:(c + 1) * BLOC], pt[:],
                    mybir.ActivationFunctionType.Identity,
                    bias=b1t[:, c:c + 1], scale=1.0,
                )

            if dbg:
                nc.sync.dma_start(curdbg_d[:], curb[:])

            # ---------------- scan constants
            w2hi_t = stage.tile([128, NCHUNK, NO], bf16, tag="w2hi")
            nc.sync.dma_start(w2hi_t[:], w2hi_d[:])
            w2lo_t = stage.tile([128, NCHUNK, NO], bf16, tag="w2lo")
            nc.sync.dma_start(w2lo_t[:], w2lo_d[:])
            b2b_t = stage.tile([128, NO], f32, tag="b2b")
            nc.sync.dma_start(b2b_t[:], b2b_d[:])
            biasm1 = stage.tile([128, 1], f32, tag="bm1")
            nc.vector.memset(biasm1[:], -1.0)
            zeros_t = stage.tile([128, NCHUNK * BLOC], f32, tag="zeros")
            nc.vector.memset(zeros_t[:], 0.0)
            z32 = stage.tile([BLOC, NO], f32, tag="z32")
            nc.vector.memset(z32[:], 0.0)
            c2r = stage.tile([BLOC, T * NO], f32, tag="c2r")
            mem2r = stage.tile([BLOC, T * NO], f32, tag="mem2r")
            spk2r = stage.tile([BLOC, T * NO], f32, tag="spk2r")

            def d_step(dt):
                """Layer-2 LIF step dt (0-based) on DVE, [32, 5]."""
                in0 = z32[:] if dt == 0 else mem2r[:, (dt - 1) * NO:dt * NO]
                nc.vector._custom_dve(
                    LIF,
                    out=mem2r[:, dt * NO:(dt + 1) * NO],
                    in0=in0,
                    in1=c2r[:, dt * NO:(dt + 1) * NO],
                    s0=BETA,
                )

            # ---------------- phase B/C/D: fused scan
            mem_prev = zeros_t
            gt = None
            for t in range(1, T + 1):
                gi, sl = (t - 1) // G, (t - 1) % G
                if sl == 0:
                    gt = gpool.tile([128, NCHUNK, G * BLOC], bf16, tag="gt")
                m = mpool.tile([128, NCHUNK * BLOC], f32, tag="m")
                nc.vector._custom_dve(LIF, out=m[:], in0=mem_prev[:], in1=curb[:], s0=BETA)
                # g = sign(mem - 1) in {-1,+1} bf16 on ACT; spk=(1+g)/2 folded
                # into the 0.5-scaled W2 and b2eff on the host.
                nc.scalar.activation(
                    gt[:, :, sl * BLOC:(sl + 1) * BLOC],
                    m[:].rearrange("p (c b) -> p c b", b=BLOC),
                    mybir.ActivationFunctionType.Sign, bias=biasm1[:], scale=1.0,
                )
                mem_prev = m
                if sl == G - 1:
                    pc = psC.tile([128, NO], f32, tag="psc")
                    for c in range(NCHUNK):
                        lhs = gt[:, c, :]
                        nc.tensor.matmul(pc[:], lhs, w2hi_t[:, c, :], start=(c == 0), stop=False)
                        nc.tensor.matmul(pc[:], lhs, w2lo_t[:, c, :], start=False,
                                         stop=(c == NCHUNK - 1))
                    # GpSimd can't read PSUM: ACT copies out, GpSimd adds b2eff
                    pcs = ppool.tile([128, NO], f32, tag="pcs")
                    nc.scalar.activation(pcs[:], pc[:],
                                         mybir.ActivationFunctionType.Copy)
                    pcb = ppool.tile([128, NO], f32, tag="pcb")
                    nc.gpsimd.tensor_tensor(
                        pcb[:], pcs[:], b2b_t[:], mybir.AluOpType.add,
                    )
                    # custom-DVE in1 can't take a partition offset: DMA each
                    # sl-row block down to partition base 0 in (t, o) layout
                    for s2 in range(G):
                        dt2 = gi * G + s2
                        nc.sync.dma_start(
                            c2r[:, dt2 * NO:(dt2 + 1) * NO],
                            pcb[s2 * BLOC:(s2 + 1) * BLOC, :],
                        )
                    if dbg and gi == 0:
                        gtf = stage.tile([128, NCHUNK * G * BLOC], f32, tag="gtf")
                        nc.vector.tensor_copy(
                            gtf[:], gt[:].rearrange("p c s -> p (c s)"))
                        nc.sync.dma_start(gtdbg_d[:], gtf[:])
                    if gi >= DLAG:
                        for dt in range((gi - DLAG) * G, (gi - DLAG + 1) * G):
                            d_step(dt)
            for dt in range((NGROUP - DLAG) * G, T):
                d_step(dt)

            # ---------------- spk2 + outputs
            if dbg:
                nc.sync.dma_start(c2dbg_d[:], c2r[:])
            nc.vector.tensor_scalar(spk2r[:], mem2r[:], 1.0, None, mybir.AluOpType.is_gt)
            nc.sync.dma_start(mem2_d[:], mem2r[:])
            nc.sync.dma_start(spk2_d[:], spk2r[:])

    nc.compile()
    _PROGRAMS[key] = (nc, LIF)
    return _PROGRAMS[key]


# ---------------------------------------------------------------- host prep
def _prep_inputs(x, W1, b1, W2, b2):
    f32 = np.float32
    x_flat = np.ascontiguousarray(x.reshape(B, -1).astype(f32, copy=False))  # [256, 32000]
    xT = np.zeros((KPAD, B), f32)
    xT[:NIN] = x_flat.T
    xTh = xT.astype(np.float16)
    xTl = (xT - xTh.astype(f32)).astype(np.float16)
    w1T = np.zeros((KPAD, HPAD), f32)
    w1T[:NIN, :NH] = W1.astype(f32, copy=False).T * W1SCALE
    w1Th = w1T.astype(np.float16)
    w1Tl = (w1T - w1Th.astype(f32)).astype(np.float16)
    b1p = np.full(HPAD, -10.0, f32)
    b1p[:NH] = b1
    b1c = np.ascontiguousarray(b1p.reshape(NCHUNK, 128).T)          # [128, 8]
    W2e = np.zeros((HPAD, NO), f32)
    W2e[:NH] = 0.5 * W2.astype(f32, copy=False).T
    w2stack = np.ascontiguousarray(W2e.reshape(NCHUNK, 128, NO).transpose(1, 0, 2))  # [128,8,5]
    w2hi = w2stack.astype(ml_dtypes.bfloat16)
    w2lo = (w2stack - w2hi.astype(f32)).astype(ml_dtypes.bfloat16)
    b2eff = (b2.astype(f32) + 0.5 * W2.astype(f32).sum(axis=1)).reshape(1, NO)
    b2b = np.ascontiguousarray(np.tile(b2eff, (128, 1)).astype(f32))

    in_maps = []
    for c in range(N_CORES):
        ksl = slice(c * KC, (c + 1) * KC)
        in_maps.append({
            "xth": np.ascontiguousarray(xTh[ksl]).reshape(KTILES, 128, B),
            "xtl": np.ascontiguousarray(xTl[ksl]).reshape(KTILES, 128, B),
            "w1h": np.ascontiguousarray(w1Th[ksl]).reshape(KTILES, 128, HPAD),
            "w1l": np.ascontiguousarray(w1Tl[ksl]).reshape(KTILES, 128, HPAD),
            "b1c": b1c,
            "w2hi": w2hi,
            "w2lo": w2lo,
            "b2b": b2b,
        })
    return in_maps


def _gather(results):
    spk_parts, mem_parts = [], []
    for r in results:
        mem_parts.append(r["mem2rec"].reshape(BLOC, T, NO).transpose(1, 0, 2))
        spk_parts.append(r["spk2rec"].reshape(BLOC, T, NO).transpose(1, 0, 2))
    mem2 = np.concatenate(mem_parts, axis=1).astype(np.float32)  # [200, 256, 5]
    spk2 = np.concatenate(spk_parts, axis=1).astype(np.float32)
    return spk2, mem2


def run_raw(inputs, dbg=False, **kwargs):
    """Build+run; returns BassKernelResults (for profiling from test.py)."""
    from concourse.bass_utils import run_bass_kernel_spmd

    nc, _ = _build_program(dbg=dbg)
    in_maps = _prep_inputs(**inputs)
    return run_bass_kernel_spmd(nc, in_maps, core_ids=list(range(N_CORES)), **kwargs)


def kernel(x, W1, b1, W2, b2):
    res = run_raw(dict(x=x, W1=W1, b1=b1, W2=W2, b2=b2))
    return _gather(res.results)


if __name__ == "__main__":
    rng = np.random.default_rng(0)
    ins = {
        "x": rng.standard_normal((B, 2, 80, 200)).astype(np.float32),
        "W1": rng.uniform(-1, 1, (NH, NIN)).astype(np.float32) / np.sqrt(NIN),
        "b1": rng.uniform(-1, 1, NH).astype(np.float32) / np.sqrt(NIN),
        "W2": rng.uniform(-1, 1, (NO, NH)).astype(np.float32) / np.sqrt(NH),
        "b2": rng.uniform(-1, 1, NO).astype(np.float32) / np.sqrt(NH),
    }
    spk2, mem2 = kernel(**ins)
    print("shapes:", spk2.shape, mem2.shape, spk2.dtype, mem2.dtype)
    print("spk2 mean:", spk2.mean(), "mem2 std:", mem2.std())


# revision 30
# speedup vs baseline: 1.2821x; 1.0621x over previous
"""Trainium2 Bass kernel for the SNN (LIF) network:

    cur1 = x.reshape(B,-1) @ W1.T + b1          (big fp32 matmul, once)
    200 sequential LIF steps on [B,1000] (layer 1), tiny matmul into 5
    outputs per step (layer 2), second LIF on [B,5].

Distribution over 8 cores (v2, scheduling-optimized):
  Phase A: contraction(K)-sharded exact-fp32 matmul (fp16 hi/lo, 3
           passes), split into two hidden halves; each half's partial
           [256, 512] goes through its own ReduceScatter(add) so the
           collective for half 0 overlaps the matmul of half 1. Each
           core ends with its 32-row batch slice of cur1.
  Phase B: per-core LIF layer-1 scan, hidden on partitions
           ([128, 8 chunks x 32 batch] tiles). One custom DVE
           instruction per step: mem' = beta*mem + cur - (mem > 1).
           Pool engine computes spk = (mem > 1) in bf16 {0,1}.
  Phase C: every 4 steps, PE contracts spk (stationary [128, 4*32])
           against W2 chunks split hi/lo bf16 (exact) into PSUM
           [128(sl,b), 5]; Pool adds b2 into cur2s.
  Phase D: layer-2 LIF steps on [32, 5], interleaved on DVE two groups
           behind phase C. spk2 = (mem2 > 1) on Pool at the end.
"""
import os
import sys

if "/opt/trn_rl_repo" not in sys.path:
    sys.path.insert(0, "/opt/trn_rl_repo")

# Profile every core when NTFF tracing is on: exec time = max per-core span
# with aligned starts, instead of core 0's span inflated by the runtime's
# per-device dispatch stagger (~2.3ms) while it waits at the collective.
os.environ.setdefault("BASS_PERFETTO_PROFILE_ALL_CORES", "1")

import numpy as np
import ml_dtypes

# ---------------------------------------------------------------- constants
BETA = 0.95
T = 200
B = 256
NIN = 32000
NH = 1000
NO = 5

N_CORES = 8
KPAD = 32768           # NIN padded to 256*128
KC = KPAD // N_CORES   # 4096 contraction per core
KTILES = KC // 128     # 32
HPAD = 1024            # hidden padded
HHALF = HPAD // 2      # 512 per pipelined half
BLOC = B // N_CORES    # 32 batch rows per core after ReduceScatter
NCHUNK = HPAD // 128   # 8 hidden chunks of 128
G = 4                  # phase-C group size (steps per PE batch)
NGROUP = T // G        # 50
GBATCH = 5             # groups per cur2 partition-shift DMA batch
W1SCALE = 256.0        # W1 pre-scale so the fp16 lo-half stays normal

# ---------------------------------------------------------------- custom op
_LIF_NAME = "LIF_STEP_ANT"


def _register_lif_op():
    from concourse.dve_ops import (
        DveOp, OPS, CUSTOM_DVE_SPECS, _SUB_OPCODE_FOR_NAME, _CUSTOM_DVE_ROW_BASE,
    )
    from concourse.dve_spec import Spec, Src0, Src1, C0, One, lower as dve_lower, _has_src1
    from concourse.dve_uop import DveOpSpec

    for op in OPS:
        if op.name == _LIF_NAME:
            return op
    spec = Spec(
        body=Src0 * C0 + Src1 - (Src0 > One),
        reference=lambda in0, in1, s0: in0 * s0 + in1 - (in0 > 1.0).astype(np.float32),
    )
    if _LIF_NAME not in _SUB_OPCODE_FOR_NAME:
        _SUB_OPCODE_FOR_NAME[_LIF_NAME] = _CUSTOM_DVE_ROW_BASE + len(OPS)
    shas = {}
    for ver in ("v3", "v4"):
        s = DveOpSpec(
            name=_LIF_NAME,
            opcode=_SUB_OPCODE_FOR_NAME[_LIF_NAME],
            uops=dve_lower(spec, ver=ver),
            rd1_en=_has_src1(spec),
        )
        shas[ver] = s.sha(ver)
    op = DveOp(_LIF_NAME, spec, subdim=False, uops_sha=shas)
    OPS.append(op)
    CUSTOM_DVE_SPECS[_LIF_NAME] = op.spec
    return op


# ---------------------------------------------------------------- program
_PROGRAMS = {}  # sim -> (nc, lif_op)


def _build_program(sim=False, dbg=False):
    key = (sim, dbg)
    if key in _PROGRAMS:
        return _PROGRAMS[key]

    import concourse.bass as bass
    import concourse.tile as tile
    from concourse import bacc, mybir
    from concourse.masks import make_identity

    LIF = _register_lif_op()
    f32 = mybir.dt.float32
    bf16 = mybir.dt.bfloat16
    f16 = mybir.dt.float16

    nc = bacc.Bacc("TRN2", target_bir_lowering=False, debug=False,
                   num_devices=1 if sim else N_CORES)

    # inputs (per-core)
    xth_d = nc.dram_tensor("xth", [KTILES, 128, B], f16, kind="ExternalInput").ap()
    xtl_d = nc.dram_tensor("xtl", [KTILES, 128, B], f16, kind="ExternalInput").ap()
    w1h_d = nc.dram_tensor("w1h", [KTILES, 128, HPAD], f16, kind="ExternalInput").ap()
    w1l_d = nc.dram_tensor("w1l", [KTILES, 128, HPAD], f16, kind="ExternalInput").ap()
    b1c_d = nc.dram_tensor("b1c", [128, NCHUNK], f32, kind="ExternalInput").ap()
    w2hi_d = nc.dram_tensor("w2hi", [128, NCHUNK, NO], bf16, kind="ExternalInput").ap()
    w2lo_d = nc.dram_tensor("w2lo", [128, NCHUNK, NO], bf16, kind="ExternalInput").ap()
    b2b_d = nc.dram_tensor("b2b", [128, NO], f32, kind="ExternalInput").ap()
    # outputs (per-core batch slice), free layout = (t, o)
    mem2_d = nc.dram_tensor("mem2rec", [BLOC, T * NO], f32, kind="ExternalOutput").ap()
    spk2_d = nc.dram_tensor("spk2rec", [BLOC, T * NO], f32, kind="ExternalOutput").ap()
    if dbg:
        curdbg_d = nc.dram_tensor("curdbg", [128, NCHUNK * BLOC], f32,
                                  kind="ExternalOutput").ap()
        c2dbg_d = nc.dram_tensor("c2dbg", [BLOC, T * NO], f32,
                                 kind="ExternalOutput").ap()
        gtdbg_d = nc.dram_tensor("gtdbg", [128, NCHUNK * G * BLOC], f32,
                                 kind="ExternalOutput").ap()

    with tile.TileContext(nc) as tc:
        with (
            tc.tile_pool(name="xres", bufs=1) as xres,
            tc.tile_pool(name="win", bufs=8) as wpool,
            tc.tile_pool(name="psA", bufs=1, space="PSUM") as psA,
            tc.tile_pool(name="stage", bufs=1) as stage,
            tc.tile_pool(name="dram", bufs=1, space="DRAM") as dram,
            tc.tile_pool(name="mem", bufs=4) as mpool,
            tc.tile_pool(name="g4", bufs=3) as gpool,
            tc.tile_pool(name="psC", bufs=2, space="PSUM") as psC,
            tc.tile_pool(name="pp", bufs=2) as ppool,
            tc.tile_pool(name="psT", bufs=2, space="PSUM") as psT,
        ):
            # ---------------- phase A, split along K for a pipelined RS
            # Full-row W DMAs ([128,1024], 256KB contiguous) keep the DMA
            # queues at peak rate; the PSUM accumulation stops at the K
            # midpoint so the first half's ReduceScatter overlaps the second
            # half's matmuls. rs = rs_a + rs_b afterwards on DVE.
            xall_h = xres.tile([128, KTILES, B], f16, tag="xah")
            xall_l = xres.tile([128, KTILES, B], f16, tag="xal")
            KSPLIT = [(0, KTILES // 2), (KTILES // 2, KTILES)]
            rs_outs = []
            for kk, (ka, kb) in enumerate(KSPLIT):
                ps = [[psA.tile([128, 512], f32, tag=f"ps{mb}{nb}",
                                name=f"ps{mb}{nb}_{kk}")
                       for nb in range(2)] for mb in range(2)]
                for kt in range(ka, kb):
                    nc.sync.dma_start(xall_h[:, kt, :], xth_d[kt])
                    nc.sync.dma_start(xall_l[:, kt, :], xtl_d[kt])
                    wh_t = wpool.tile([128, HPAD], f16, tag="w1h")
                    nc.sync.dma_start(wh_t[:], w1h_d[kt])
                    wl_t = wpool.tile([128, HPAD], f16, tag="w1l")
                    nc.sync.dma_start(wl_t[:], w1l_d[kt])
                    last = kt == kb - 1
                    for mb in range(2):
                        xh_s = xall_h[:, kt, mb * 128:(mb + 1) * 128]
                        xl_s = xall_l[:, kt, mb * 128:(mb + 1) * 128]
                        # keep each stationary operand loaded across streams
                        for nb in range(2):
                            out = ps[mb][nb][:]
                            nc.tensor.matmul(out, xh_s, wl_t[:, nb * 512:(nb + 1) * 512],
                                             start=(kt == ka), stop=False)
                            nc.tensor.matmul(out, xh_s, wh_t[:, nb * 512:(nb + 1) * 512],
                                             start=False, stop=False)
                        for nb in range(2):
                            nc.tensor.matmul(ps[mb][nb][:], xl_s,
                                             wh_t[:, nb * 512:(nb + 1) * 512],
                                             start=False, stop=last)
                partial = dram.tile([B, HPAD], f32, tag=f"partial{kk}",
                                    name=f"partial{kk}")
                for mb in range(2):
                    cs = stage.tile([128, HPAD], f32, tag=f"curp{mb}",
                                    name=f"cs{mb}{kk}")
                    for nb in range(2):
                        nc.scalar.activation(
                            cs[:, nb * 512:(nb + 1) * 512], ps[mb][nb][:],
                            mybir.ActivationFunctionType.Copy, scale=1.0 / W1SCALE)
                    # chunk the DMA across queues to cut drain latency
                    for q in range(4):
                        nc.sync.dma_start(
                            partial[mb * 128:(mb + 1) * 128, q * 256:(q + 1) * 256],
                            cs[:, q * 256:(q + 1) * 256])
                rs_out = dram.tile([BLOC, HPAD], f32, tag=f"rs{kk}", name=f"rs{kk}")
                if sim:
                    nc.sync.dma_start(rs_out[:], partial[0:BLOC, :])
                else:
                    nc.gpsimd.collective_compute(
                        "ReduceScatter",
                        mybir.AluOpType.add,
                        replica_groups=[list(range(N_CORES))],
                        ins=[partial.opt()],
                        outs=[rs_out.opt()],
                    )
                rs_outs.append(rs_out)

            # ---------------- transpose to scan layout + fold b1
            # curb[p, c*32 + b] = cur1[b, c*128 + p] + b1[c*128 + p]
            ident = stage.tile([BLOC, BLOC], f32, tag="ident")
            make_identity(nc, ident[:])
            b1t = stage.tile([128, NCHUNK], f32, tag="b1t")
            nc.sync.dma_start(b1t[:], b1c_d[:])
            rsb = [stage.tile([BLOC, HPAD], f32, tag=f"rsb{kk}", name=f"rsb{kk}")
                   for kk in range(2)]
            for kk in range(2):
                for q in range(4):
                    nc.sync.dma_start(rsb[kk][:, q * 256:(q + 1) * 256],
                                      rs_outs[kk][:, q * 256:(q + 1) * 256])
            curb = stage.tile([128, NCHUNK * BLOC], f32, tag="curb")
            for c in range(NCHUNK):
                # transpose both K-half slices into one accumulating PSUM tile
                pt = psT.tile([128, BLOC], f32, tag="pst")
                nc.tensor.matmul(pt[:], rsb[0][:, c * 128:(c + 1) * 128], ident[:],
                                 start=True, stop=False, is_transpose=True)
                nc.tensor.matmul(pt[:], rsb[1][:, c * 128:(c + 1) * 128], ident[:],
                                 start=False, stop=True, is_transpose=True)
                nc.scalar.activation(
                    curb[:, c * BLOC:(c + 1) * BLOC], pt[:],
                    mybir.ActivationFunctionType.Identity,
                    bias=b1t[:, c:c + 1], scale=1.0,
                )

            if dbg:
                nc.sync.dma_start(curdbg_d[:], curb[:])

            # ---------------- scan constants
            w2hi_t = stage.tile([128, NCHUNK, NO], bf16, tag="w2hi")
            nc.sync.dma_start(w2hi_t[:], w2hi_d[:])
            w2lo_t = stage.tile([128, NCHUNK, NO], bf16, tag="w2lo")
            nc.sync.dma_start(w2lo_t[:], w2lo_d[:])
            b2b_t = stage.tile([128, NO], f32, tag="b2b")
            nc.sync.dma_start(b2b_t[:], b2b_d[:])
            biasm1 = stage.tile([128, 1], f32, tag="bm1")
            nc.vector.memset(biasm1[:], -1.0)
            zeros_t = stage.tile([128, NCHUNK * BLOC], f32, tag="zeros")
            nc.vector.memset(zeros_t[:], 0.0)
            z32 = stage.tile([BLOC, NO], f32, tag="z32")
            nc.vector.memset(z32[:], 0.0)
            c2r = stage.tile([BLOC, T * NO], f32, tag="c2r")
            mem2r = stage.tile([BLOC, T * NO], f32, tag="mem2r")
            spk2r = stage.tile([BLOC, T * NO], f32, tag="spk2r")

            def d_step(dt):
                """Layer-2 LIF step dt (0-based) on DVE, [32, 5]."""
                in0 = z32[:] if dt == 0 else mem2r[:, (dt - 1) * NO:dt * NO]
                nc.vector._custom_dve(
                    LIF,
                    out=mem2r[:, dt * NO:(dt + 1) * NO],
                    in0=in0,
                    in1=c2r[:, dt * NO:(dt + 1) * NO],
                    s0=BETA,
                )

            # ---------------- phase B/C/D: fused scan
            mem_prev = zeros_t
            gt = None
            for t in range(1, T + 1):
                gi, sl = (t - 1) // G, (t - 1) % G
                if sl == 0:
                    gt = gpool.tile([128, NCHUNK, G * BLOC], bf16, tag="gt")
                m = mpool.tile([128, NCHUNK * BLOC], f32, tag="m")
                nc.vector._custom_dve(LIF, out=m[:], in0=mem_prev[:], in1=curb[:], s0=BETA)
                # g = sign(mem - 1) in {-1,+1} bf16 on ACT; spk=(1+g)/2 folded
                # into the 0.5-scaled W2 and b2eff on the host.
                nc.scalar.activation(
                    gt[:, :, sl * BLOC:(sl + 1) * BLOC],
                    m[:].rearrange("p (c b) -> p c b", b=BLOC),
                    mybir.ActivationFunctionType.Sign, bias=biasm1[:], scale=1.0,
                )
                mem_prev = m
                if sl == G - 1:
                    if gi % GBATCH == 0:
                        pcbB = ppool.tile([128, GBATCH, NO], f32, tag="pcbB")
                    pc = psC.tile([128, NO], f32, tag="psc")
                    for c in range(NCHUNK):
                        lhs = gt[:, c, :]
                        nc.tensor.matmul(pc[:], lhs, w2hi_t[:, c, :], start=(c == 0), stop=False)
                        nc.tensor.matmul(pc[:], lhs, w2lo_t[:, c, :], start=False,
                                         stop=(c == NCHUNK - 1))
                    # GpSimd can't read PSUM: ACT copies out, GpSimd adds b2eff
                    pcs = ppool.tile([128, NO], f32, tag="pcs")
                    nc.scalar.activation(pcs[:], pc[:],
                                         mybir.ActivationFunctionType.Copy)
                    nc.gpsimd.tensor_tensor(
                        pcbB[:, gi % GBATCH, :], pcs[:], b2b_t[:],
                        mybir.AluOpType.add,
                    )
                    if gi % GBATCH == GBATCH - 1:
                        # custom-DVE in1 can't take a partition offset: DMA the
                        # batched sl-row blocks down to partition base 0 in
                        # (t, o) layout. Batching 5 groups per DMA keeps the
                        # SP sequencer's 580ns-per-DMA cost off the scan.
                        gb = gi // GBATCH
                        bview = c2r[:, gb * GBATCH * G * NO:(gb + 1) * GBATCH * G * NO]
                        bview = bview.rearrange("b (g s o) -> b g s o", s=G, o=NO)
                        for s2 in range(G):
                            nc.sync.dma_start(
                                bview[:, :, s2, :],
                                pcbB[s2 * BLOC:(s2 + 1) * BLOC, :, :],
                            )
                        if gb >= 1:
                            for dt in range((gb - 1) * GBATCH * G, gb * GBATCH * G):
                                d_step(dt)
                    if dbg and gi == 0:
                        gtf = stage.tile([128, NCHUNK * G * BLOC], f32, tag="gtf")
                        nc.vector.tensor_copy(
                            gtf[:], gt[:].rearrange("p c s -> p (c s)"))
                        nc.sync.dma_start(gtdbg_d[:], gtf[:])
            for dt in range((NGROUP // GBATCH - 1) * GBATCH * G, T):
                d_step(dt)

            # ---------------- spk2 + outputs
            if dbg:
                nc.sync.dma_start(c2dbg_d[:], c2r[:])
            nc.vector.tensor_scalar(spk2r[:], mem2r[:], 1.0, None, mybir.AluOpType.is_gt)
            nc.sync.dma_start(mem2_d[:], mem2r[:])
            nc.sync.dma_start(spk2_d[:], spk2r[:])

    nc.compile()
    _PROGRAMS[key] = (nc, LIF)
    return _PROGRAMS[key]


# ---------------------------------------------------------------- host prep
def _prep_inputs(x, W1, b1, W2, b2):
    f32 = np.float32
    x_flat = np.ascontiguousarray(x.reshape(B, -1).astype(f32, copy=False))  # [256, 32000]
    xT = np.zeros((KPAD, B), f32)
    xT[:NIN] = x_flat.T
    xTh = xT.astype(np.float16)
    xTl = (xT - xTh.astype(f32)).astype(np.float16)
    w1T = np.zeros((KPAD, HPAD), f32)
    w1T[:NIN, :NH] = W1.astype(f32, copy=False).T * W1SCALE
    w1Th = w1T.astype(np.float16)
    w1Tl = (w1T - w1Th.astype(f32)).astype(np.float16)
    b1p = np.full(HPAD, -10.0, f32)
    b1p[:NH] = b1
    b1c = np.ascontiguousarray(b1p.reshape(NCHUNK, 128).T)          # [128, 8]
    W2e = np.zeros((HPAD, NO), f32)
    W2e[:NH] = 0.5 * W2.astype(f32, copy=False).T
    w2stack = np.ascontiguousarray(W2e.reshape(NCHUNK, 128, NO).transpose(1, 0, 2))  # [128,8,5]
    w2hi = w2stack.astype(ml_dtypes.bfloat16)
    w2lo = (w2stack - w2hi.astype(f32)).astype(ml_dtypes.bfloat16)
    b2eff = (b2.astype(f32) + 0.5 * W2.astype(f32).sum(axis=1)).reshape(1, NO)
    b2b = np.ascontiguousarray(np.tile(b2eff, (128, 1)).astype(f32))

    in_maps = []
    for c in range(N_CORES):
        ksl = slice(c * KC, (c + 1) * KC)
        in_maps.append({
            "xth": np.ascontiguousarray(xTh[ksl]).reshape(KTILES, 128, B),
            "xtl": np.ascontiguousarray(xTl[ksl]).reshape(KTILES, 128, B),
            "w1h": np.ascontiguousarray(w1Th[ksl]).reshape(KTILES, 128, HPAD),
            "w1l": np.ascontiguousarray(w1Tl[ksl]).reshape(KTILES, 128, HPAD),
            "b1c": b1c,
            "w2hi": w2hi,
            "w2lo": w2lo,
            "b2b": b2b,
        })
    return in_maps


def _gather(results):
    spk_parts, mem_parts = [], []
    for r in results:
        mem_parts.append(r["mem2rec"].reshape(BLOC, T, NO).transpose(1, 0, 2))
        spk_parts.append(r["spk2rec"].reshape(BLOC, T, NO).transpose(1, 0, 2))
    mem2 = np.concatenate(mem_parts, axis=1).astype(np.float32)  # [200, 256, 5]
    spk2 = np.concatenate(spk_parts, axis=1).astype(np.float32)
    return spk2, mem2


def run_raw(inputs, dbg=False, **kwargs):
    """Build+run; returns BassKernelResults (for profiling from test.py)."""
    from concourse.bass_utils import run_bass_kernel_spmd

    nc, _ = _build_program(dbg=dbg)
    in_maps = _prep_inputs(**inputs)
    return run_bass_kernel_spmd(nc, in_maps, core_ids=list(range(N_CORES)), **kwargs)


def kernel(x, W1, b1, W2, b2):
    res = run_raw(dict(x=x, W1=W1, b1=b1, W2=W2, b2=b2))
    return _gather(res.results)


if __name__ == "__main__":
    rng = np.random.default_rng(0)
    ins = {
        "x": rng.standard_normal((B, 2, 80, 200)).astype(np.float32),
        "W1": rng.uniform(-1, 1, (NH, NIN)).astype(np.float32) / np.sqrt(NIN),
        "b1": rng.uniform(-1, 1, NH).astype(np.float32) / np.sqrt(NIN),
        "W2": rng.uniform(-1, 1, (NO, NH)).astype(np.float32) / np.sqrt(NH),
        "b2": rng.uniform(-1, 1, NO).astype(np.float32) / np.sqrt(NH),
    }
    spk2, mem2 = kernel(**ins)
    print("shapes:", spk2.shape, mem2.shape, spk2.dtype, mem2.dtype)
    print("spk2 mean:", spk2.mean(), "mem2 std:", mem2.std())


# revision 35
# speedup vs baseline: 1.2838x; 1.0014x over previous
"""Trainium2 Bass kernel for the SNN (LIF) network:

    cur1 = x.reshape(B,-1) @ W1.T + b1          (big fp32 matmul, once)
    200 sequential LIF steps on [B,1000] (layer 1), tiny matmul into 5
    outputs per step (layer 2), second LIF on [B,5].

Distribution over 8 cores (v2, scheduling-optimized):
  Phase A: contraction(K)-sharded exact-fp32 matmul (fp16 hi/lo, 3
           passes), split into two hidden halves; each half's partial
           [256, 512] goes through its own ReduceScatter(add) so the
           collective for half 0 overlaps the matmul of half 1. Each
           core ends with its 32-row batch slice of cur1.
  Phase B: per-core LIF layer-1 scan, hidden on partitions
           ([128, 8 chunks x 32 batch] tiles). One custom DVE
           instruction per step: mem' = beta*mem + cur - (mem > 1).
           Pool engine computes spk = (mem > 1) in bf16 {0,1}.
  Phase C: every 4 steps, PE contracts spk (stationary [128, 4*32])
           against W2 chunks split hi/lo bf16 (exact) into PSUM
           [128(sl,b), 5]; Pool adds b2 into cur2s.
  Phase D: layer-2 LIF steps on [32, 5], interleaved on DVE two groups
           behind phase C. spk2 = (mem2 > 1) on Pool at the end.
"""
import os
import sys

if "/opt/trn_rl_repo" not in sys.path:
    sys.path.insert(0, "/opt/trn_rl_repo")

# Profile every core when NTFF tracing is on: exec time = max per-core span
# with aligned starts, instead of core 0's span inflated by the runtime's
# per-device dispatch stagger (~2.3ms) while it waits at the collective.
os.environ.setdefault("BASS_PERFETTO_PROFILE_ALL_CORES", "1")

import numpy as np
import ml_dtypes

# ---------------------------------------------------------------- constants
BETA = 0.95
T = 200
B = 256
NIN = 32000
NH = 1000
NO = 5

N_CORES = 8
KPAD = 32768           # NIN padded to 256*128
KC = KPAD // N_CORES   # 4096 contraction per core
KTILES = KC // 128     # 32
HPAD = 1024            # hidden padded
HHALF = HPAD // 2      # 512 per pipelined half
BLOC = B // N_CORES    # 32 batch rows per core after ReduceScatter
NCHUNK = HPAD // 128   # 8 hidden chunks of 128
G = 4                  # phase-C group size (steps per PE batch)
NGROUP = T // G        # 50
GBATCH = 5             # groups per cur2 partition-shift DMA batch
W1SCALE = 256.0        # W1 pre-scale so the fp16 lo-half stays normal

# ---------------------------------------------------------------- custom op
_LIF_NAME = "LIF_STEP_ANT"


def _register_lif_op():
    from concourse.dve_ops import (
        DveOp, OPS, CUSTOM_DVE_SPECS, _SUB_OPCODE_FOR_NAME, _CUSTOM_DVE_ROW_BASE,
    )
    from concourse.dve_spec import Spec, Src0, Src1, C0, One, lower as dve_lower, _has_src1
    from concourse.dve_uop import DveOpSpec

    for op in OPS:
        if op.name == _LIF_NAME:
            return op
    spec = Spec(
        body=Src0 * C0 + Src1 - (Src0 > One),
        reference=lambda in0, in1, s0: in0 * s0 + in1 - (in0 > 1.0).astype(np.float32),
    )
    if _LIF_NAME not in _SUB_OPCODE_FOR_NAME:
        _SUB_OPCODE_FOR_NAME[_LIF_NAME] = _CUSTOM_DVE_ROW_BASE + len(OPS)
    shas = {}
    for ver in ("v3", "v4"):
        s = DveOpSpec(
            name=_LIF_NAME,
            opcode=_SUB_OPCODE_FOR_NAME[_LIF_NAME],
            uops=dve_lower(spec, ver=ver),
            rd1_en=_has_src1(spec),
        )
        shas[ver] = s.sha(ver)
    op = DveOp(_LIF_NAME, spec, subdim=False, uops_sha=shas)
    OPS.append(op)
    CUSTOM_DVE_SPECS[_LIF_NAME] = op.spec
    return op


# ---------------------------------------------------------------- program
_PROGRAMS = {}  # sim -> (nc, lif_op)


def _build_program(sim=False, dbg=False):
    key = (sim, dbg)
    if key in _PROGRAMS:
        return _PROGRAMS[key]

    import concourse.bass as bass
    import concourse.tile as tile
    from concourse import bacc, mybir
    from concourse.masks import make_identity

    LIF = _register_lif_op()
    f32 = mybir.dt.float32
    bf16 = mybir.dt.bfloat16
    f16 = mybir.dt.float16

    nc = bacc.Bacc("TRN2", target_bir_lowering=False, debug=False,
                   num_devices=1 if sim else N_CORES)

    # inputs (per-core)
    xth_d = nc.dram_tensor("xth", [KTILES, 128, B], f16, kind="ExternalInput").ap()
    xtl_d = nc.dram_tensor("xtl", [KTILES, 128, B], f16, kind="ExternalInput").ap()
    w1h_d = nc.dram_tensor("w1h", [KTILES, 128, HPAD], f16, kind="ExternalInput").ap()
    w1l_d = nc.dram_tensor("w1l", [KTILES, 128, HPAD], f16, kind="ExternalInput").ap()
    b1c_d = nc.dram_tensor("b1c", [128, NCHUNK], f32, kind="ExternalInput").ap()
    w2hi_d = nc.dram_tensor("w2hi", [128, NCHUNK, NO], bf16, kind="ExternalInput").ap()
    w2lo_d = nc.dram_tensor("w2lo", [128, NCHUNK, NO], bf16, kind="ExternalInput").ap()
    b2b_d = nc.dram_tensor("b2b", [128, NO], f32, kind="ExternalInput").ap()
    # outputs (per-core batch slice), free layout = (t, o)
    mem2_d = nc.dram_tensor("mem2rec", [BLOC, T * NO], f32, kind="ExternalOutput").ap()
    spk2_d = nc.dram_tensor("spk2rec", [BLOC, T * NO], f32, kind="ExternalOutput").ap()
    if dbg:
        curdbg_d = nc.dram_tensor("curdbg", [128, NCHUNK * BLOC], f32,
                                  kind="ExternalOutput").ap()
        c2dbg_d = nc.dram_tensor("c2dbg", [BLOC, T * NO], f32,
                                 kind="ExternalOutput").ap()
        gtdbg_d = nc.dram_tensor("gtdbg", [128, NCHUNK * G * BLOC], f32,
                                 kind="ExternalOutput").ap()

    with tile.TileContext(nc) as tc:
        with (
            tc.tile_pool(name="xres", bufs=1) as xres,
            tc.tile_pool(name="win", bufs=3) as wpool,
            tc.tile_pool(name="psA", bufs=1, space="PSUM") as psA,
            tc.tile_pool(name="stage", bufs=1) as stage,
            tc.tile_pool(name="dram", bufs=1, space="DRAM") as dram,
            tc.tile_pool(name="mem", bufs=6) as mpool,
            tc.tile_pool(name="g4", bufs=4) as gpool,
            tc.tile_pool(name="psC", bufs=2, space="PSUM") as psC,
            tc.tile_pool(name="pp", bufs=2) as ppool,
            tc.tile_pool(name="psT", bufs=2, space="PSUM") as psT,
        ):
            # ---------------- phase A, split along K for a pipelined RS
            # Full-row W DMAs ([128,1024], 256KB contiguous) keep the DMA
            # queues at peak rate; the PSUM accumulation stops at the K
            # midpoint so the first half's ReduceScatter overlaps the second
            # half's matmuls. rs = rs_a + rs_b afterwards on DVE.
            xall_h = xres.tile([128, KTILES, B], f16, tag="xah")
            xall_l = xres.tile([128, KTILES, B], f16, tag="xal")
            KSPLIT = [(0, KTILES // 2), (KTILES // 2, KTILES)]
            rs_outs = []
            for kk, (ka, kb) in enumerate(KSPLIT):
                ps = [[psA.tile([128, 512], f32, tag=f"ps{mb}{nb}",
                                name=f"ps{mb}{nb}_{kk}")
                       for nb in range(2)] for mb in range(2)]
                # 4-kt quads per DMA: each dma_start costs ~580ns of SP
                # sequencer issue time, so fewer/bigger transfers win.
                KQ = 4
                for kt0 in range(ka, kb, KQ):
                    nc.sync.dma_start(
                        xall_h[:, kt0:kt0 + KQ, :],
                        xth_d[kt0:kt0 + KQ].rearrange("k p b -> p k b"))
                    nc.sync.dma_start(
                        xall_l[:, kt0:kt0 + KQ, :],
                        xtl_d[kt0:kt0 + KQ].rearrange("k p b -> p k b"))
                    wh_t = wpool.tile([128, KQ, HPAD], f16, tag="w1h")
                    nc.sync.dma_start(wh_t[:], w1h_d[kt0:kt0 + KQ].rearrange("k p n -> p k n"))
                    wl_t = wpool.tile([128, KQ, HPAD], f16, tag="w1l")
                    nc.sync.dma_start(wl_t[:], w1l_d[kt0:kt0 + KQ].rearrange("k p n -> p k n"))
                    for kq in range(KQ):
                        kt = kt0 + kq
                        last = kt == kb - 1
                        for mb in range(2):
                            xh_s = xall_h[:, kt, mb * 128:(mb + 1) * 128]
                            xl_s = xall_l[:, kt, mb * 128:(mb + 1) * 128]
                            # keep each stationary operand loaded across streams
                            for nb in range(2):
                                out = ps[mb][nb][:]
                                nc.tensor.matmul(out, xh_s,
                                                 wl_t[:, kq, nb * 512:(nb + 1) * 512],
                                                 start=(kt == ka), stop=False)
                                nc.tensor.matmul(out, xh_s,
                                                 wh_t[:, kq, nb * 512:(nb + 1) * 512],
                                                 start=False, stop=False)
                            for nb in range(2):
                                nc.tensor.matmul(ps[mb][nb][:], xl_s,
                                                 wh_t[:, kq, nb * 512:(nb + 1) * 512],
                                                 start=False, stop=last)
                partial = dram.tile([B, HPAD], f32, tag=f"partial{kk}",
                                    name=f"partial{kk}")
                for mb in range(2):
                    cs = stage.tile([128, HPAD], f32, tag=f"curp{mb}",
                                    name=f"cs{mb}{kk}")
                    for nb in range(2):
                        nc.scalar.activation(
                            cs[:, nb * 512:(nb + 1) * 512], ps[mb][nb][:],
                            mybir.ActivationFunctionType.Copy, scale=1.0 / W1SCALE)
                    # chunk the DMA across queues to cut drain latency
                    for q in range(4):
                        nc.sync.dma_start(
                            partial[mb * 128:(mb + 1) * 128, q * 256:(q + 1) * 256],
                            cs[:, q * 256:(q + 1) * 256])
                rs_out = dram.tile([BLOC, HPAD], f32, tag=f"rs{kk}", name=f"rs{kk}")
                if sim:
                    nc.sync.dma_start(rs_out[:], partial[0:BLOC, :])
                else:
                    nc.gpsimd.collective_compute(
                        "ReduceScatter",
                        mybir.AluOpType.add,
                        replica_groups=[list(range(N_CORES))],
                        ins=[partial.opt()],
                        outs=[rs_out.opt()],
                    )
                rs_outs.append(rs_out)

            # ---------------- transpose to scan layout + fold b1
            # curb[p, c*32 + b] = cur1[b, c*128 + p] + b1[c*128 + p]
            ident = stage.tile([BLOC, BLOC], f32, tag="ident")
            make_identity(nc, ident[:])
            b1t = stage.tile([128, NCHUNK], f32, tag="b1t")
            nc.sync.dma_start(b1t[:], b1c_d[:])
            rsb = [stage.tile([BLOC, HPAD], f32, tag=f"rsb{kk}", name=f"rsb{kk}")
                   for kk in range(2)]
            for kk in range(2):
                for q in range(4):
                    nc.sync.dma_start(rsb[kk][:, q * 256:(q + 1) * 256],
                                      rs_outs[kk][:, q * 256:(q + 1) * 256])
            curb = stage.tile([128, NCHUNK * BLOC], f32, tag="curb")
            for c in range(NCHUNK):
                # transpose both K-half slices into one accumulating PSUM tile
                pt = psT.tile([128, BLOC], f32, tag="pst")
                nc.tensor.matmul(pt[:], rsb[0][:, c * 128:(c + 1) * 128], ident[:],
                                 start=True, stop=False, is_transpose=True)
                nc.tensor.matmul(pt[:], rsb[1][:, c * 128:(c + 1) * 128], ident[:],
                                 start=False, stop=True, is_transpose=True)
                nc.scalar.activation(
                    curb[:, c * BLOC:(c + 1) * BLOC], pt[:],
                    mybir.ActivationFunctionType.Identity,
                    bias=b1t[:, c:c + 1], scale=1.0,
                )

            if dbg:
                nc.sync.dma_start(curdbg_d[:], curb[:])

            # ---------------- scan constants
            w2hi_t = stage.tile([128, NCHUNK, NO], bf16, tag="w2hi")
            nc.sync.dma_start(w2hi_t[:], w2hi_d[:])
            w2lo_t = stage.tile([128, NCHUNK, NO], bf16, tag="w2lo")
            nc.sync.dma_start(w2lo_t[:], w2lo_d[:])
            b2b_t = stage.tile([128, NO], f32, tag="b2b")
            nc.sync.dma_start(b2b_t[:], b2b_d[:])
            biasm1 = stage.tile([128, 1], f32, tag="bm1")
            nc.vector.memset(biasm1[:], -1.0)
            zeros_t = stage.tile([128, NCHUNK * BLOC], f32, tag="zeros")
            nc.vector.memset(zeros_t[:], 0.0)
            z32 = stage.tile([BLOC, NO], f32, tag="z32")
            nc.vector.memset(z32[:], 0.0)
            c2r = stage.tile([BLOC, T * NO], f32, tag="c2r")
            mem2r = stage.tile([BLOC, T * NO], f32, tag="mem2r")
            spk2r = stage.tile([BLOC, T * NO], f32, tag="spk2r")

            def d_step(dt):
                """Layer-2 LIF step dt (0-based) on DVE, [32, 5]."""
                in0 = z32[:] if dt == 0 else mem2r[:, (dt - 1) * NO:dt * NO]
                nc.vector._custom_dve(
                    LIF,
                    out=mem2r[:, dt * NO:(dt + 1) * NO],
                    in0=in0,
                    in1=c2r[:, dt * NO:(dt + 1) * NO],
                    s0=BETA,
                )

            # ---------------- phase B/C/D: fused scan
            mem_prev = zeros_t
            gt = None
            for t in range(1, T + 1):
                gi, sl = (t - 1) // G, (t - 1) % G
                if sl == 0:
                    gt = gpool.tile([128, NCHUNK, G * BLOC], bf16, tag="gt")
                m = mpool.tile([128, NCHUNK * BLOC], f32, tag="m")
                nc.vector._custom_dve(LIF, out=m[:], in0=mem_prev[:], in1=curb[:], s0=BETA)
                # g = sign(mem - 1) in {-1,+1} bf16 on ACT; spk=(1+g)/2 folded
                # into the 0.5-scaled W2 and b2eff on the host.
                nc.scalar.activation(
                    gt[:, :, sl * BLOC:(sl + 1) * BLOC],
                    m[:].rearrange("p (c b) -> p c b", b=BLOC),
                    mybir.ActivationFunctionType.Sign, bias=biasm1[:], scale=1.0,
                )
                mem_prev = m
                if sl == G - 1:
                    if gi % GBATCH == 0:
                        pcbB = ppool.tile([128, GBATCH, NO], f32, tag="pcbB")
                    pc = psC.tile([128, NO], f32, tag="psc")
                    for c in range(NCHUNK):
                        lhs = gt[:, c, :]
                        nc.tensor.matmul(pc[:], lhs, w2hi_t[:, c, :], start=(c == 0), stop=False)
                        nc.tensor.matmul(pc[:], lhs, w2lo_t[:, c, :], start=False,
                                         stop=(c == NCHUNK - 1))
                    # GpSimd can't read PSUM: ACT copies out, GpSimd adds b2eff
                    pcs = ppool.tile([128, NO], f32, tag="pcs")
                    nc.scalar.activation(pcs[:], pc[:],
                                         mybir.ActivationFunctionType.Copy)
                    nc.gpsimd.tensor_tensor(
                        pcbB[:, gi % GBATCH, :], pcs[:], b2b_t[:],
                        mybir.AluOpType.add,
                    )
                    if gi % GBATCH == GBATCH - 1:
                        # custom-DVE in1 can't take a partition offset: DMA the
                        # batched sl-row blocks down to partition base 0 in
                        # (t, o) layout. Batching 5 groups per DMA keeps the
                        # SP sequencer's 580ns-per-DMA cost off the scan.
                        gb = gi // GBATCH
                        bview = c2r[:, gb * GBATCH * G * NO:(gb + 1) * GBATCH * G * NO]
                        bview = bview.rearrange("b (g s o) -> b g s o", s=G, o=NO)
                        for s2 in range(G):
                            nc.sync.dma_start(
                                bview[:, :, s2, :],
                                pcbB[s2 * BLOC:(s2 + 1) * BLOC, :, :],
                            )
                        if gb >= 1:
                            b0 = (gb - 1) * GBATCH * G * NO
                            b1 = gb * GBATCH * G * NO
                            for dt in range((gb - 1) * GBATCH * G, gb * GBATCH * G):
                                d_step(dt)
                            # stream this batch's outputs out now
                            nc.vector.tensor_scalar(
                                spk2r[:, b0:b1], mem2r[:, b0:b1], 1.0, None,
                                mybir.AluOpType.is_gt)
                            nc.sync.dma_start(mem2_d[:, b0:b1], mem2r[:, b0:b1])
                            nc.sync.dma_start(spk2_d[:, b0:b1], spk2r[:, b0:b1])
                    if dbg and gi == 0:
                        gtf = stage.tile([128, NCHUNK * G * BLOC], f32, tag="gtf")
                        nc.vector.tensor_copy(
                            gtf[:], gt[:].rearrange("p c s -> p (c s)"))
                        nc.sync.dma_start(gtdbg_d[:], gtf[:])
            # final d-batch + its outputs
            b0 = (NGROUP // GBATCH - 1) * GBATCH * G * NO
            for dt in range((NGROUP // GBATCH - 1) * GBATCH * G, T):
                d_step(dt)
            if dbg:
                nc.sync.dma_start(c2dbg_d[:], c2r[:])
            nc.vector.tensor_scalar(spk2r[:, b0:], mem2r[:, b0:], 1.0, None,
                                    mybir.AluOpType.is_gt)
            nc.sync.dma_start(mem2_d[:, b0:], mem2r[:, b0:])
            nc.sync.dma_start(spk2_d[:, b0:], spk2r[:, b0:])

    nc.compile()
    _PROGRAMS[key] = (nc, LIF)
    return _PROGRAMS[key]


# ---------------------------------------------------------------- host prep
def _prep_inputs(x, W1, b1, W2, b2):
    f32 = np.float32
    x_flat = np.ascontiguousarray(x.reshape(B, -1).astype(f32, copy=False))  # [256, 32000]
    xT = np.zeros((KPAD, B), f32)
    xT[:NIN] = x_flat.T
    xTh = xT.astype(np.float16)
    xTl = (xT - xTh.astype(f32)).astype(np.float16)
    w1T = np.zeros((KPAD, HPAD), f32)
    w1T[:NIN, :NH] = W1.astype(f32, copy=False).T * W1SCALE
    w1Th = w1T.astype(np.float16)
    w1Tl = (w1T - w1Th.astype(f32)).astype(np.float16)
    b1p = np.full(HPAD, -10.0, f32)
    b1p[:NH] = b1
    b1c = np.ascontiguousarray(b1p.reshape(NCHUNK, 128).T)          # [128, 8]
    W2e = np.zeros((HPAD, NO), f32)
    W2e[:NH] = 0.5 * W2.astype(f32, copy=False).T
    w2stack = np.ascontiguousarray(W2e.reshape(NCHUNK, 128, NO).transpose(1, 0, 2))  # [128,8,5]
    w2hi = w2stack.astype(ml_dtypes.bfloat16)
    w2lo = (w2stack - w2hi.astype(f32)).astype(ml_dtypes.bfloat16)
    b2eff = (b2.astype(f32) + 0.5 * W2.astype(f32).sum(axis=1)).reshape(1, NO)
    b2b = np.ascontiguousarray(np.tile(b2eff, (128, 1)).astype(f32))

    in_maps = []
    for c in range(N_CORES):
        ksl = slice(c * KC, (c + 1) * KC)
        in_maps.append({
            "xth": np.ascontiguousarray(xTh[ksl]).reshape(KTILES, 128, B),
            "xtl": np.ascontiguousarray(xTl[ksl]).reshape(KTILES, 128, B),
            "w1h": np.ascontiguousarray(w1Th[ksl]).reshape(KTILES, 128, HPAD),
            "w1l": np.ascontiguousarray(w1Tl[ksl]).reshape(KTILES, 128, HPAD),
            "b1c": b1c,
            "w2hi": w2hi,
            "w2lo": w2lo,
            "b2b": b2b,
        })
    return in_maps


def _gather(results):
    spk_parts, mem_parts = [], []
    for r in results:
        mem_parts.append(r["mem2rec"].reshape(BLOC, T, NO).transpose(1, 0, 2))
        spk_parts.append(r["spk2rec"].reshape(BLOC, T, NO).transpose(1, 0, 2))
    mem2 = np.concatenate(mem_parts, axis=1).astype(np.float32)  # [200, 256, 5]
    spk2 = np.concatenate(spk_parts, axis=1).astype(np.float32)
    return spk2, mem2


def run_raw(inputs, dbg=False, **kwargs):
    """Build+run; returns BassKernelResults (for profiling from test.py)."""
    from concourse.bass_utils import run_bass_kernel_spmd

    nc, _ = _build_program(dbg=dbg)
    in_maps = _prep_inputs(**inputs)
    return run_bass_kernel_spmd(nc, in_maps, core_ids=list(range(N_CORES)), **kwargs)


def kernel(x, W1, b1, W2, b2):
    res = run_raw(dict(x=x, W1=W1, b1=b1, W2=W2, b2=b2))
    return _gather(res.results)


if __name__ == "__main__":
    rng = np.random.default_rng(0)
    ins = {
        "x": rng.standard_normal((B, 2, 80, 200)).astype(np.float32),
        "W1": rng.uniform(-1, 1, (NH, NIN)).astype(np.float32) / np.sqrt(NIN),
        "b1": rng.uniform(-1, 1, NH).astype(np.float32) / np.sqrt(NIN),
        "W2": rng.uniform(-1, 1, (NO, NH)).astype(np.float32) / np.sqrt(NH),
        "b2": rng.uniform(-1, 1, NO).astype(np.float32) / np.sqrt(NH),
    }
    spk2, mem2 = kernel(**ins)
    print("shapes:", spk2.shape, mem2.shape, spk2.dtype, mem2.dtype)
    print("spk2 mean:", spk2.mean(), "mem2 std:", mem2.std())


# revision 37
# speedup vs baseline: 1.3024x; 1.0144x over previous
"""Trainium2 Bass kernel for the SNN (LIF) network:

    cur1 = x.reshape(B,-1) @ W1.T + b1          (big fp32 matmul, once)
    200 sequential LIF steps on [B,1000] (layer 1), tiny matmul into 5
    outputs per step (layer 2), second LIF on [B,5].

Distribution over 8 cores (v2, scheduling-optimized):
  Phase A: contraction(K)-sharded exact-fp32 matmul (fp16 hi/lo, 3
           passes), split into two hidden halves; each half's partial
           [256, 512] goes through its own ReduceScatter(add) so the
           collective for half 0 overlaps the matmul of half 1. Each
           core ends with its 32-row batch slice of cur1.
  Phase B: per-core LIF layer-1 scan, hidden on partitions
           ([128, 8 chunks x 32 batch] tiles). One custom DVE
           instruction per step: mem' = beta*mem + cur - (mem > 1).
           Pool engine computes spk = (mem > 1) in bf16 {0,1}.
  Phase C: every 4 steps, PE contracts spk (stationary [128, 4*32])
           against W2 chunks split hi/lo bf16 (exact) into PSUM
           [128(sl,b), 5]; Pool adds b2 into cur2s.
  Phase D: layer-2 LIF steps on [32, 5], interleaved on DVE two groups
           behind phase C. spk2 = (mem2 > 1) on Pool at the end.
"""
import os
import sys

if "/opt/trn_rl_repo" not in sys.path:
    sys.path.insert(0, "/opt/trn_rl_repo")

# Profile every core when NTFF tracing is on: exec time = max per-core span
# with aligned starts, instead of core 0's span inflated by the runtime's
# per-device dispatch stagger (~2.3ms) while it waits at the collective.
os.environ.setdefault("BASS_PERFETTO_PROFILE_ALL_CORES", "1")

import numpy as np
import ml_dtypes

# ---------------------------------------------------------------- constants
BETA = 0.95
T = 200
B = 256
NIN = 32000
NH = 1000
NO = 5

N_CORES = 8
KPAD = 32768           # NIN padded to 256*128
KC = KPAD // N_CORES   # 4096 contraction per core
KTILES = KC // 128     # 32
HPAD = 1024            # hidden padded
HHALF = HPAD // 2      # 512 per pipelined half
BLOC = B // N_CORES    # 32 batch rows per core after ReduceScatter
NCHUNK = HPAD // 128   # 8 hidden chunks of 128
G = 4                  # phase-C group size (steps per PE batch)
NGROUP = T // G        # 50
GBATCH = 2             # groups per cur2 partition-shift DMA batch
W1SCALE = 256.0        # W1 pre-scale so the fp16 lo-half stays normal

# ---------------------------------------------------------------- custom op
_LIF_NAME = "LIF_STEP_ANT"


def _register_lif_op():
    from concourse.dve_ops import (
        DveOp, OPS, CUSTOM_DVE_SPECS, _SUB_OPCODE_FOR_NAME, _CUSTOM_DVE_ROW_BASE,
    )
    from concourse.dve_spec import Spec, Src0, Src1, C0, One, lower as dve_lower, _has_src1
    from concourse.dve_uop import DveOpSpec

    for op in OPS:
        if op.name == _LIF_NAME:
            return op
    spec = Spec(
        body=Src0 * C0 + Src1 - (Src0 > One),
        reference=lambda in0, in1, s0: in0 * s0 + in1 - (in0 > 1.0).astype(np.float32),
    )
    if _LIF_NAME not in _SUB_OPCODE_FOR_NAME:
        _SUB_OPCODE_FOR_NAME[_LIF_NAME] = _CUSTOM_DVE_ROW_BASE + len(OPS)
    shas = {}
    for ver in ("v3", "v4"):
        s = DveOpSpec(
            name=_LIF_NAME,
            opcode=_SUB_OPCODE_FOR_NAME[_LIF_NAME],
            uops=dve_lower(spec, ver=ver),
            rd1_en=_has_src1(spec),
        )
        shas[ver] = s.sha(ver)
    op = DveOp(_LIF_NAME, spec, subdim=False, uops_sha=shas)
    OPS.append(op)
    CUSTOM_DVE_SPECS[_LIF_NAME] = op.spec
    return op


# ---------------------------------------------------------------- program
_PROGRAMS = {}  # sim -> (nc, lif_op)


def _build_program(sim=False, dbg=False):
    key = (sim, dbg)
    if key in _PROGRAMS:
        return _PROGRAMS[key]

    import concourse.bass as bass
    import concourse.tile as tile
    from concourse import bacc, mybir
    from concourse.masks import make_identity

    LIF = _register_lif_op()
    f32 = mybir.dt.float32
    bf16 = mybir.dt.bfloat16
    f16 = mybir.dt.float16

    nc = bacc.Bacc("TRN2", target_bir_lowering=False, debug=False,
                   num_devices=1 if sim else N_CORES)

    # inputs (per-core)
    xth_d = nc.dram_tensor("xth", [KTILES, 128, B], f16, kind="ExternalInput").ap()
    xtl_d = nc.dram_tensor("xtl", [KTILES, 128, B], f16, kind="ExternalInput").ap()
    w1h_d = nc.dram_tensor("w1h", [KTILES, 128, HPAD], f16, kind="ExternalInput").ap()
    w1l_d = nc.dram_tensor("w1l", [KTILES, 128, HPAD], f16, kind="ExternalInput").ap()
    b1c_d = nc.dram_tensor("b1c", [128, NCHUNK], f32, kind="ExternalInput").ap()
    w2hi_d = nc.dram_tensor("w2hi", [128, NCHUNK, NO], bf16, kind="ExternalInput").ap()
    w2lo_d = nc.dram_tensor("w2lo", [128, NCHUNK, NO], bf16, kind="ExternalInput").ap()
    b2b_d = nc.dram_tensor("b2b", [128, NO], f32, kind="ExternalInput").ap()
    # outputs (per-core batch slice), free layout = (t, o)
    mem2_d = nc.dram_tensor("mem2rec", [BLOC, T * NO], f32, kind="ExternalOutput").ap()
    spk2_d = nc.dram_tensor("spk2rec", [BLOC, T * NO], f32, kind="ExternalOutput").ap()
    if dbg:
        curdbg_d = nc.dram_tensor("curdbg", [128, NCHUNK * BLOC], f32,
                                  kind="ExternalOutput").ap()
        c2dbg_d = nc.dram_tensor("c2dbg", [BLOC, T * NO], f32,
                                 kind="ExternalOutput").ap()
        gtdbg_d = nc.dram_tensor("gtdbg", [128, NCHUNK * G * BLOC], f32,
                                 kind="ExternalOutput").ap()

    with tile.TileContext(nc) as tc:
        with (
            tc.tile_pool(name="xres", bufs=1) as xres,
            tc.tile_pool(name="win", bufs=3) as wpool,
            tc.tile_pool(name="psA", bufs=1, space="PSUM") as psA,
            tc.tile_pool(name="stage", bufs=1) as stage,
            tc.tile_pool(name="dram", bufs=1, space="DRAM") as dram,
            tc.tile_pool(name="mem", bufs=6) as mpool,
            tc.tile_pool(name="g4", bufs=4) as gpool,
            tc.tile_pool(name="psC", bufs=2, space="PSUM") as psC,
            tc.tile_pool(name="pp", bufs=2) as ppool,
            tc.tile_pool(name="psT", bufs=2, space="PSUM") as psT,
        ):
            # ---------------- phase A, split along K for a pipelined RS
            # Full-row W DMAs ([128,1024], 256KB contiguous) keep the DMA
            # queues at peak rate; the PSUM accumulation stops at the K
            # midpoint so the first half's ReduceScatter overlaps the second
            # half's matmuls. rs = rs_a + rs_b afterwards on DVE.
            xall_h = xres.tile([128, KTILES, B], f16, tag="xah")
            xall_l = xres.tile([128, KTILES, B], f16, tag="xal")
            KSPLIT = [(0, KTILES // 2), (KTILES // 2, KTILES)]
            rs_outs = []
            for kk, (ka, kb) in enumerate(KSPLIT):
                ps = [[psA.tile([128, 512], f32, tag=f"ps{mb}{nb}",
                                name=f"ps{mb}{nb}_{kk}")
                       for nb in range(2)] for mb in range(2)]
                # 4-kt quads per DMA: each dma_start costs ~580ns of SP
                # sequencer issue time, so fewer/bigger transfers win.
                KQ = 4
                for kt0 in range(ka, kb, KQ):
                    wh_t = wpool.tile([128, KQ, HPAD], f16, tag="w1h")
                    wl_t = wpool.tile([128, KQ, HPAD], f16, tag="w1l")
                    if kk == 0 and kt0 == 0:
                        # per-kt transfers so the first matmul isn't gated on
                        # a 1MB quad landing
                        for j in range(KQ):
                            nc.sync.dma_start(xall_h[:, j, :], xth_d[j])
                            nc.sync.dma_start(xall_l[:, j, :], xtl_d[j])
                            nc.sync.dma_start(wh_t[:, j, :], w1h_d[j])
                            nc.sync.dma_start(wl_t[:, j, :], w1l_d[j])
                    else:
                        nc.sync.dma_start(
                            xall_h[:, kt0:kt0 + KQ, :],
                            xth_d[kt0:kt0 + KQ].rearrange("k p b -> p k b"))
                        nc.sync.dma_start(
                            xall_l[:, kt0:kt0 + KQ, :],
                            xtl_d[kt0:kt0 + KQ].rearrange("k p b -> p k b"))
                        nc.sync.dma_start(wh_t[:], w1h_d[kt0:kt0 + KQ].rearrange("k p n -> p k n"))
                        nc.sync.dma_start(wl_t[:], w1l_d[kt0:kt0 + KQ].rearrange("k p n -> p k n"))
                    for kq in range(KQ):
                        kt = kt0 + kq
                        last = kt == kb - 1
                        for mb in range(2):
                            xh_s = xall_h[:, kt, mb * 128:(mb + 1) * 128]
                            xl_s = xall_l[:, kt, mb * 128:(mb + 1) * 128]
                            # keep each stationary operand loaded across streams
                            for nb in range(2):
                                out = ps[mb][nb][:]
                                nc.tensor.matmul(out, xh_s,
                                                 wl_t[:, kq, nb * 512:(nb + 1) * 512],
                                                 start=(kt == ka), stop=False)
                                nc.tensor.matmul(out, xh_s,
                                                 wh_t[:, kq, nb * 512:(nb + 1) * 512],
                                                 start=False, stop=False)
                            for nb in range(2):
                                nc.tensor.matmul(ps[mb][nb][:], xl_s,
                                                 wh_t[:, kq, nb * 512:(nb + 1) * 512],
                                                 start=False, stop=last)
                partial = dram.tile([B, HPAD], f32, tag=f"partial{kk}",
                                    name=f"partial{kk}")
                for mb in range(2):
                    cs = stage.tile([128, HPAD], f32, tag=f"curp{mb}",
                                    name=f"cs{mb}{kk}")
                    for nb in range(2):
                        nc.scalar.activation(
                            cs[:, nb * 512:(nb + 1) * 512], ps[mb][nb][:],
                            mybir.ActivationFunctionType.Copy, scale=1.0 / W1SCALE)
                    # chunk the DMA across queues to cut drain latency
                    for q in range(4):
                        nc.sync.dma_start(
                            partial[mb * 128:(mb + 1) * 128, q * 256:(q + 1) * 256],
                            cs[:, q * 256:(q + 1) * 256])
                rs_out = dram.tile([BLOC, HPAD], f32, tag=f"rs{kk}", name=f"rs{kk}")
                if sim:
                    nc.sync.dma_start(rs_out[:], partial[0:BLOC, :])
                else:
                    nc.gpsimd.collective_compute(
                        "ReduceScatter",
                        mybir.AluOpType.add,
                        replica_groups=[list(range(N_CORES))],
                        ins=[partial.opt()],
                        outs=[rs_out.opt()],
                    )
                rs_outs.append(rs_out)

            # ---------------- transpose to scan layout + fold b1
            # curb[p, c*32 + b] = cur1[b, c*128 + p] + b1[c*128 + p]
            ident = stage.tile([BLOC, BLOC], f32, tag="ident")
            make_identity(nc, ident[:])
            b1t = stage.tile([128, NCHUNK], f32, tag="b1t")
            nc.sync.dma_start(b1t[:], b1c_d[:])
            rsb = [stage.tile([BLOC, HPAD], f32, tag=f"rsb{kk}", name=f"rsb{kk}")
                   for kk in range(2)]
            for kk in range(2):
                for q in range(4):
                    nc.sync.dma_start(rsb[kk][:, q * 256:(q + 1) * 256],
                                      rs_outs[kk][:, q * 256:(q + 1) * 256])
            curb = stage.tile([128, NCHUNK * BLOC], f32, tag="curb")
            for c in range(NCHUNK):
                # transpose both K-half slices into one accumulating PSUM tile
                pt = psT.tile([128, BLOC], f32, tag="pst")
                nc.tensor.matmul(pt[:], rsb[0][:, c * 128:(c + 1) * 128], ident[:],
                                 start=True, stop=False, is_transpose=True)
                nc.tensor.matmul(pt[:], rsb[1][:, c * 128:(c + 1) * 128], ident[:],
                                 start=False, stop=True, is_transpose=True)
                nc.scalar.activation(
                    curb[:, c * BLOC:(c + 1) * BLOC], pt[:],
                    mybir.ActivationFunctionType.Identity,
                    bias=b1t[:, c:c + 1], scale=1.0,
                )

            if dbg:
                nc.sync.dma_start(curdbg_d[:], curb[:])

            # ---------------- scan constants
            w2hi_t = stage.tile([128, NCHUNK, NO], bf16, tag="w2hi")
            nc.sync.dma_start(w2hi_t[:], w2hi_d[:])
            w2lo_t = stage.tile([128, NCHUNK, NO], bf16, tag="w2lo")
            nc.sync.dma_start(w2lo_t[:], w2lo_d[:])
            b2b_t = stage.tile([128, NO], f32, tag="b2b")
            nc.sync.dma_start(b2b_t[:], b2b_d[:])
            biasm1 = stage.tile([128, 1], f32, tag="bm1")
            nc.vector.memset(biasm1[:], -1.0)
            zeros_t = stage.tile([128, NCHUNK * BLOC], f32, tag="zeros")
            nc.vector.memset(zeros_t[:], 0.0)
            z32 = stage.tile([BLOC, NO], f32, tag="z32")
            nc.vector.memset(z32[:], 0.0)
            c2r = stage.tile([BLOC, T * NO], f32, tag="c2r")
            mem2r = stage.tile([BLOC, T * NO], f32, tag="mem2r")
            spk2r = stage.tile([BLOC, T * NO], f32, tag="spk2r")

            def d_step(dt):
                """Layer-2 LIF step dt (0-based) on DVE, [32, 5]."""
                in0 = z32[:] if dt == 0 else mem2r[:, (dt - 1) * NO:dt * NO]
                nc.vector._custom_dve(
                    LIF,
                    out=mem2r[:, dt * NO:(dt + 1) * NO],
                    in0=in0,
                    in1=c2r[:, dt * NO:(dt + 1) * NO],
                    s0=BETA,
                )

            # ---------------- phase B/C/D: fused scan
            mem_prev = zeros_t
            gt = None
            for t in range(1, T + 1):
                gi, sl = (t - 1) // G, (t - 1) % G
                if sl == 0:
                    gt = gpool.tile([128, NCHUNK, G * BLOC], bf16, tag="gt")
                m = mpool.tile([128, NCHUNK * BLOC], f32, tag="m")
                nc.vector._custom_dve(LIF, out=m[:], in0=mem_prev[:], in1=curb[:], s0=BETA)
                # g = sign(mem - 1) in {-1,+1} bf16 on ACT; spk=(1+g)/2 folded
                # into the 0.5-scaled W2 and b2eff on the host.
                nc.scalar.activation(
                    gt[:, :, sl * BLOC:(sl + 1) * BLOC],
                    m[:].rearrange("p (c b) -> p c b", b=BLOC),
                    mybir.ActivationFunctionType.Sign, bias=biasm1[:], scale=1.0,
                )
                mem_prev = m
                if sl == G - 1:
                    if gi % GBATCH == 0:
                        pcbB = ppool.tile([128, GBATCH, NO], f32, tag="pcbB")
                    pc = psC.tile([128, NO], f32, tag="psc")
                    for c in range(NCHUNK):
                        lhs = gt[:, c, :]
                        nc.tensor.matmul(pc[:], lhs, w2hi_t[:, c, :], start=(c == 0), stop=False)
                        nc.tensor.matmul(pc[:], lhs, w2lo_t[:, c, :], start=False,
                                         stop=(c == NCHUNK - 1))
                    # GpSimd can't read PSUM: ACT copies out, GpSimd adds b2eff
                    pcs = ppool.tile([128, NO], f32, tag="pcs")
                    nc.scalar.activation(pcs[:], pc[:],
                                         mybir.ActivationFunctionType.Copy)
                    nc.gpsimd.tensor_tensor(
                        pcbB[:, gi % GBATCH, :], pcs[:], b2b_t[:],
                        mybir.AluOpType.add,
                    )
                    if gi % GBATCH == GBATCH - 1:
                        # custom-DVE in1 can't take a partition offset: DMA the
                        # batched sl-row blocks down to partition base 0 in
                        # (t, o) layout. Batching 5 groups per DMA keeps the
                        # SP sequencer's 580ns-per-DMA cost off the scan.
                        gb = gi // GBATCH
                        bview = c2r[:, gb * GBATCH * G * NO:(gb + 1) * GBATCH * G * NO]
                        bview = bview.rearrange("b (g s o) -> b g s o", s=G, o=NO)
                        for s2 in range(G):
                            nc.sync.dma_start(
                                bview[:, :, s2, :],
                                pcbB[s2 * BLOC:(s2 + 1) * BLOC, :, :],
                            )
                        if gb >= 1:
                            b0 = (gb - 1) * GBATCH * G * NO
                            b1 = gb * GBATCH * G * NO
                            for dt in range((gb - 1) * GBATCH * G, gb * GBATCH * G):
                                d_step(dt)
                            # stream this batch's outputs out now
                            nc.vector.tensor_scalar(
                                spk2r[:, b0:b1], mem2r[:, b0:b1], 1.0, None,
                                mybir.AluOpType.is_gt)
                            nc.sync.dma_start(mem2_d[:, b0:b1], mem2r[:, b0:b1])
                            nc.sync.dma_start(spk2_d[:, b0:b1], spk2r[:, b0:b1])
                    if dbg and gi == 0:
                        gtf = stage.tile([128, NCHUNK * G * BLOC], f32, tag="gtf")
                        nc.vector.tensor_copy(
                            gtf[:], gt[:].rearrange("p c s -> p (c s)"))
                        nc.sync.dma_start(gtdbg_d[:], gtf[:])
            # final d-batch + its outputs
            b0 = (NGROUP // GBATCH - 1) * GBATCH * G * NO
            for dt in range((NGROUP // GBATCH - 1) * GBATCH * G, T):
                d_step(dt)
            if dbg:
                nc.sync.dma_start(c2dbg_d[:], c2r[:])
            nc.vector.tensor_scalar(spk2r[:, b0:], mem2r[:, b0:], 1.0, None,
                                    mybir.AluOpType.is_gt)
            nc.sync.dma_start(mem2_d[:, b0:], mem2r[:, b0:])
            nc.sync.dma_start(spk2_d[:, b0:], spk2r[:, b0:])

    nc.compile()
    _PROGRAMS[key] = (nc, LIF)
    return _PROGRAMS[key]


# ---------------------------------------------------------------- host prep
def _prep_inputs(x, W1, b1, W2, b2):
    f32 = np.float32
    x_flat = np.ascontiguousarray(x.reshape(B, -1).astype(f32, copy=False))  # [256, 32000]
    xT = np.zeros((KPAD, B), f32)
    xT[:NIN] = x_flat.T
    xTh = xT.astype(np.float16)
    xTl = (xT - xTh.astype(f32)).astype(np.float16)
    w1T = np.zeros((KPAD, HPAD), f32)
    w1T[:NIN, :NH] = W1.astype(f32, copy=False).T * W1SCALE
    w1Th = w1T.astype(np.float16)
    w1Tl = (w1T - w1Th.astype(f32)).astype(np.float16)
    b1p = np.full(HPAD, -10.0, f32)
    b1p[:NH] = b1
    b1c = np.ascontiguousarray(b1p.reshape(NCHUNK, 128).T)          # [128, 8]
    W2e = np.zeros((HPAD, NO), f32)
    W2e[:NH] = 0.5 * W2.astype(f32, copy=False).T
    w2stack = np.ascontiguousarray(W2e.reshape(NCHUNK, 128, NO).transpose(1, 0, 2))  # [128,8,5]
    w2hi = w2stack.astype(ml_dtypes.bfloat16)
    w2lo = (w2stack - w2hi.astype(f32)).astype(ml_dtypes.bfloat16)
    b2eff = (b2.astype(f32) + 0.5 * W2.astype(f32).sum(axis=1)).reshape(1, NO)
    b2b = np.ascontiguousarray(np.tile(b2eff, (128, 1)).astype(f32))

    in_maps = []
    for c in range(N_CORES):
        ksl = slice(c * KC, (c + 1) * KC)
        in_maps.append({
            "xth": np.ascontiguousarray(xTh[ksl]).reshape(KTILES, 128, B),
            "xtl": np.ascontiguousarray(xTl[ksl]).reshape(KTILES, 128, B),
            "w1h": np.ascontiguousarray(w1Th[ksl]).reshape(KTILES, 128, HPAD),
            "w1l": np.ascontiguousarray(w1Tl[ksl]).reshape(KTILES, 128, HPAD),
            "b1c": b1c,
            "w2hi": w2hi,
            "w2lo": w2lo,
            "b2b": b2b,
        })
    return in_maps


def _gather(results):
    spk_parts, mem_parts = [], []
    for r in results:
        mem_parts.append(r["mem2rec"].reshape(BLOC, T, NO).transpose(1, 0, 2))
        spk_parts.append(r["spk2rec"].reshape(BLOC, T, NO).transpose(1, 0, 2))
    mem2 = np.concatenate(mem_parts, axis=1).astype(np.float32)  # [200, 256, 5]
    spk2 = np.concatenate(spk_parts, axis=1).astype(np.float32)
    return spk2, mem2


def run_raw(inputs, dbg=False, **kwargs):
    """Build+run; returns BassKernelResults (for profiling from test.py)."""
    from concourse.bass_utils import run_bass_kernel_spmd

    nc, _ = _build_program(dbg=dbg)
    in_maps = _prep_inputs(**inputs)
    return run_bass_kernel_spmd(nc, in_maps, core_ids=list(range(N_CORES)), **kwargs)


def kernel(x, W1, b1, W2, b2):
    res = run_raw(dict(x=x, W1=W1, b1=b1, W2=W2, b2=b2))
    return _gather(res.results)


if __name__ == "__main__":
    rng = np.random.default_rng(0)
    ins = {
        "x": rng.standard_normal((B, 2, 80, 200)).astype(np.float32),
        "W1": rng.uniform(-1, 1, (NH, NIN)).astype(np.float32) / np.sqrt(NIN),
        "b1": rng.uniform(-1, 1, NH).astype(np.float32) / np.sqrt(NIN),
        "W2": rng.uniform(-1, 1, (NO, NH)).astype(np.float32) / np.sqrt(NH),
        "b2": rng.uniform(-1, 1, NO).astype(np.float32) / np.sqrt(NH),
    }
    spk2, mem2 = kernel(**ins)
    print("shapes:", spk2.shape, mem2.shape, spk2.dtype, mem2.dtype)
    print("spk2 mean:", spk2.mean(), "mem2 std:", mem2.std())
